# revision 44
# baseline (speedup 1.0000x reference)
"""AttentionBlock (GroupNorm -> QKV 1x1conv -> 2304x2304 spatial attention)
on 8 Trainium2 NeuronCores, data-parallel over batch.

Per core (one batch element b), with x = input[b] viewed as [C=256, N=2304]:
  gn    = groupnorm(x)                          (32 groups of 8 channels)
  q,k   = Wq@gn, Wk@gn        stored [C, N]     (channel on partitions)
  vT    = gn^T @ Wv^T (+ ones col)  [N, C+1]    (pixel on partitions)
  S^T   = k^T tiles . q       [n-tile, m]       (scores transposed)
  E     = exp(S^T / 16)                         (no max-sub: |scores| ~ 1)
  out   = (vT^T . E) / (ones^T . E)             (rowsum via vT's ones col)

All large matmuls run fp32r (full PE rate at free-dim >= 256).
"""
import numpy as np
from contextlib import ExitStack

import concourse.bass as bass
import concourse.tile as tile
from concourse import bacc, mybir
from concourse.bass_utils import run_bass_kernel_spmd

B, C, HH, WW = 8, 256, 48, 48
NPIX = HH * WW            # 2304
G = 32                    # groups
CPG = C // G              # 8 channels per group
EPS = 1e-5
N_CORES = 8
NT = NPIX // 128          # 18 pixel tiles
SLAB = 512
M_SLABS = [(j, min(SLAB, NPIX - j)) for j in range(0, NPIX, SLAB)]
VTW = C                   # vT slab width (no ones col; see onescol)
INV_GN = 1.0 / (CPG * NPIX)
QK_SCALE = 1.0 / 16.0     # 1/sqrt(C)

F32 = mybir.dt.float32
F32R = mybir.dt.float32r
AF = mybir.ActivationFunctionType
ALU = mybir.AluOpType
AX = mybir.AxisListType

_NC_CACHE = []


def _emit(nc, tc, d):
    with ExitStack() as ctx:
        pers = ctx.enter_context(tc.tile_pool(name="pers", bufs=1))

        # ---- persistent SBUF tensors ----
        x_t = [pers.tile([128, NPIX], F32, name=f"x{t}") for t in range(2)]
        xn_t = [pers.tile([128, NPIX], F32R, name=f"xn{t}") for t in range(2)]
        q_t = [pers.tile([128, NPIX], F32R, name=f"q{t}") for t in range(2)]
        k_t = [pers.tile([128, NPIX], F32R, name=f"k{t}") for t in range(2)]
        vt_all = pers.tile([128, NT * VTW], F32R, name="vt_all")

        wq = [pers.tile([128, C], F32R, name=f"wq{t}") for t in range(2)]
        wk = [pers.tile([128, C], F32R, name=f"wk{t}") for t in range(2)]
        wv = [pers.tile([128, VTW], F32R, name=f"wv{t}") for t in range(2)]
        vbias = pers.tile([128, VTW], F32, name="vbias")
        bq2 = pers.tile([128, 2], F32, name="bq2")
        bk2 = pers.tile([128, 2], F32, name="bk2")
        gnw2 = pers.tile([128, 2], F32, name="gnw2")
        gnb2 = pers.tile([128, 2], F32, name="gnb2")
        memb = [pers.tile([128, 128], F32, name=f"memb{t}") for t in range(2)]
        membT = [pers.tile([G, 128], F32, name=f"membT{t}") for t in range(2)]
        ones_r = pers.tile([1, 128], F32R, name="ones_r")
        onescol = pers.tile([128, 128], F32R, name="onescol")

        # Everything waits on GN stats, which need the full x: give x the
        # whole HBM bandwidth first (one tile per DMA queue), then the tiny
        # GN constants, then the projection weights (needed a few us later).
        HNP = NPIX // 2
        for h in range(2):
            nc.sync.dma_start(
                x_t[0][:, h * HNP:(h + 1) * HNP],
                d["x"][0:128, h * HNP:(h + 1) * HNP])
            nc.scalar.dma_start(
                x_t[1][:, h * HNP:(h + 1) * HNP],
                d["x"][128:256, h * HNP:(h + 1) * HNP])
        for t in range(2):
            nc.sync.dma_start(memb[t][:], d[f"memb{t}"][:])
            nc.scalar.dma_start(membT[t][:], d[f"membT{t}"][:])
        nc.sync.dma_start(gnw2[:], d["gnw2"][:])
        nc.scalar.dma_start(gnb2[:], d["gnb2"][:])
        for t in range(2):
            nc.sync.dma_start(wq[t][:], d["wqT"][t * 128:(t + 1) * 128, :])
            nc.scalar.dma_start(wk[t][:], d["wkT"][t * 128:(t + 1) * 128, :])
            nc.sync.dma_start(wv[t][:], d["wvT"][t * 128:(t + 1) * 128, :])
        nc.scalar.dma_start(vbias[:], d["vbias"][:])
        nc.sync.dma_start(bq2[:], d["bq2"][:])
        nc.scalar.dma_start(bk2[:], d["bk2"][:])
        nc.sync.dma_start(ones_r[:], d["ones_r"][:])
        nc.scalar.dma_start(onescol[:], d["onescol"][:])

        # ================= GroupNorm =================
        with ExitStack() as pctx:
            sc = pctx.enter_context(tc.tile_pool(name="gn_sc", bufs=2))
            gps = pctx.enter_context(
                tc.tile_pool(name="gn_ps", bufs=2, space="PSUM"))

            # per-partition mean/var in one DVE pass per half via bn_stats
            # (384-wide windows; 2304 = 6 x 384), aggregated by bn_aggr.
            # stats[t] = [mean_ch, E[x^2]_ch]
            BNW = 384
            stats = [sc.tile([128, 2], F32, name=f"stats{t}") for t in range(2)]
            for t in range(2):
                bnraw = sc.tile([128, 36], F32, name="bnraw", tag="bnr",
                                bufs=2)
                for w in range(6):
                    nc.vector.bn_stats(
                        bnraw[:, 6 * w:6 * w + 6],
                        x_t[t][:, w * BNW:(w + 1) * BNW])
                mv = sc.tile([128, 2], F32, name="mv", tag="mv", bufs=2)
                nc.vector.bn_aggr(mv[:], bnraw[:])
                m2 = sc.tile([128, 1], F32, name="m2", tag="m2", bufs=2)
                nc.vector.tensor_mul(m2[:], mv[:, 0:1], mv[:, 0:1])
                nc.vector.tensor_copy(stats[t][:, 0:1], mv[:, 0:1])
                nc.vector.tensor_add(stats[t][:, 1:2], mv[:, 1:2], m2[:])

            # group sums: partitions 0..31 of memb^T stats (memb cols 32..127
            # are zero padding -- fp32 matmul needs full col groups)
            g_ps = gps.tile([128, 2], F32, name="g_ps")
            for t in range(2):
                nc.tensor.matmul(g_ps[:], memb[t][:], stats[t][:],
                                 start=(t == 0), stop=(t == 1))

            mr = sc.tile([G, 2], F32, name="mr")          # [mean, rstd]
            tmp1 = sc.tile([G, 1], F32, name="tmp1")
            tmp2 = sc.tile([G, 1], F32, name="tmp2")
            tmp3 = sc.tile([G, 1], F32, name="tmp3")
            nc.vector.tensor_scalar_mul(mr[:, 0:1], g_ps[0:G, 0:1], 1.0 / CPG)
            nc.vector.tensor_scalar_mul(tmp1[:], g_ps[0:G, 1:2], 1.0 / CPG)
            nc.vector.tensor_mul(tmp2[:], mr[:, 0:1], mr[:, 0:1])
            nc.vector.tensor_sub(tmp3[:], tmp1[:], tmp2[:])       # var
            nc.vector.tensor_scalar_add(tmp3[:], tmp3[:], EPS)
            nc.scalar.activation(tmp1[:], tmp3[:], AF.Sqrt)
            nc.vector.reciprocal_approx_fast(mr[:, 1:2], tmp1[:])  # rstd

            # broadcast per-group (mean, rstd) back to channels; fold gn w/b
            ab = []
            for t in range(2):
                ch_ps = gps.tile([128, 2], F32, name="ch_ps", tag="ch", bufs=2)
                nc.tensor.matmul(ch_ps[:], membT[t][:], mr[:],
                                 start=True, stop=True)
                a_sb = sc.tile([128, 1], F32, name="a_sb", tag="a", bufs=2)
                t_sb = sc.tile([128, 1], F32, name="t_sb", tag="t", bufs=2)
                b_sb = sc.tile([128, 1], F32, name="b_sb", tag="b", bufs=2)
                nc.vector.tensor_mul(a_sb[:], ch_ps[:, 1:2], gnw2[:, t:t + 1])
                nc.vector.tensor_mul(t_sb[:], ch_ps[:, 0:1], a_sb[:])
                nc.vector.tensor_sub(b_sb[:], gnb2[:, t:t + 1], t_sb[:])
                ab.append((a_sb, b_sb))
            # normalize slab-major so slab-0 projections start early
            for (m0, mw) in M_SLABS:
                for t in range(2):
                    nc.vector.tensor_scalar(xn_t[t][:, m0:m0 + mw],
                                            x_t[t][:, m0:m0 + mw],
                                            ab[t][0][:], ab[t][1][:],
                                            op0=ALU.mult, op1=ALU.add)

        # ================= Q/K/V projections =================
        with ExitStack() as pctx:
            pps = pctx.enter_context(
                tc.tile_pool(name="proj_ps", bufs=2, space="PSUM"))

            for (m0, mw) in M_SLABS:
                for to in range(2):          # output-channel tile
                    for (w2, b2, dst) in ((wq, bq2, q_t), (wk, bk2, k_t)):
                        ps = pps.tile([128, SLAB], F32, name="qk_ps",
                                      tag="qk", bufs=2)
                        for tch in range(2):
                            nc.tensor.matmul(
                                ps[:, :mw],
                                w2[tch][:, to * 128:(to + 1) * 128],
                                xn_t[tch][:, m0:m0 + mw],
                                start=(tch == 0), stop=(tch == 1))
                        nc.vector.tensor_scalar_add(
                            dst[to][:, m0:m0 + mw], ps[:, :mw],
                            b2[:, to:to + 1])

            for tn in range(NT):
                ps = pps.tile([128, VTW], F32, name="vt_ps", tag="vt", bufs=2)
                for tch in range(2):
                    nc.tensor.matmul(
                        ps[:],
                        xn_t[tch][:, tn * 128:(tn + 1) * 128],
                        wv[tch][:],
                        start=(tch == 0), stop=(tch == 1))
                nc.vector.tensor_add(
                    vt_all[:, tn * VTW:(tn + 1) * VTW], ps[:], vbias[:])

        # ================= attention main loop =================
        with ExitStack() as mctx:
            stp = mctx.enter_context(
                tc.tile_pool(name="st_ps", bufs=4, space="PSUM"))
            outp = mctx.enter_context(
                tc.tile_pool(name="out_ps", bufs=2, space="PSUM"))
            rowp = mctx.enter_context(
                tc.tile_pool(name="row_ps", bufs=1, space="PSUM"))
            bcp = mctx.enter_context(
                tc.tile_pool(name="bc_ps", bufs=1, space="PSUM"))
            epool = mctx.enter_context(tc.tile_pool(name="e_sb", bufs=4))
            osb = mctx.enter_context(tc.tile_pool(name="o_sb", bufs=3))
            orawp = mctx.enter_context(tc.tile_pool(name="oraw_sb", bufs=4))
            rsb = mctx.enter_context(tc.tile_pool(name="r_sb", bufs=2))

            PRE = 4  # S^T matmul prefill depth (= st_ps bufs)
            pending_epi = [None]

            for (m0, mw) in M_SLABS:
                st = {}

                def emit_s(t, m0=m0, mw=mw, st=st):
                    st[t] = stp.tile([128, SLAB], F32, name="st", tag="st",
                                     bufs=PRE)
                    for tch in range(2):
                        nc.tensor.matmul(
                            st[t][:, :mw],
                            k_t[tch][:, t * 128:(t + 1) * 128],
                            q_t[tch][:, m0:m0 + mw],
                            start=(tch == 0), stop=(tch == 1))

                oc = [outp.tile([128, SLAB], F32, name=f"oc{c}", tag="oc",
                                bufs=2) for c in range(2)]
                rows = rowp.tile([128, SLAB], F32, name="rows")

                for t in range(PRE):
                    emit_s(t)
                # previous slab's epilogue lands after this slab's S prefill
                # so the PE never sits on the denominator chain
                if pending_epi[0] is not None:
                    pending_epi[0]()
                    pending_epi[0] = None

                for t in range(NT):
                    e_t = epool.tile([128, SLAB], F32R, name="e_t", tag="e",
                                     bufs=4)
                    nc.scalar.activation(e_t[:, :mw], st[t][:, :mw], AF.Exp,
                                         scale=QK_SCALE)
                    st.pop(t)
                    if t + PRE < NT:
                        emit_s(t + PRE)
                    tn_off = t * VTW
                    for c in range(2):
                        nc.tensor.matmul(
                            oc[c][:, :mw],
                            vt_all[:, tn_off + c * 128:tn_off + (c + 1) * 128],
                            e_t[:, :mw],
                            start=(t == 0), stop=(t == NT - 1))
                    # rowsum: lhsT = [ones, 0, ..., 0] (fp32r needs M=128);
                    # the sum lands in output partition 0
                    nc.tensor.matmul(
                        rows[:, :mw],
                        onescol[:],
                        e_t[:, :mw],
                        start=(t == 0), stop=(t == NT - 1))

                # free PSUM banks immediately: raw sums -> SBUF
                oraw = []
                for c in range(2):
                    orw = orawp.tile([128, SLAB], F32, name="oraw",
                                     tag="oraw", bufs=4)
                    nc.vector.tensor_copy(orw[:, :mw], oc[c][:, :mw])
                    oraw.append(orw)
                rs_sb = rsb.tile([1, SLAB], F32R, name="rs_sb", tag="rs",
                                 bufs=2)
                nc.vector.tensor_copy(rs_sb[:, :mw], rows[0:1, :mw])

                def epilogue(oraw=oraw, rs_sb=rs_sb, m0=m0, mw=mw):
                    # denominators: broadcast rowsum to 128 partitions (K=1
                    # fp32 matmul), then fast reciprocal
                    bc = bcp.tile([128, SLAB], F32, name="bc")
                    nc.tensor.matmul(bc[:, :mw], ones_r[:], rs_sb[:, :mw],
                                     start=True, stop=True)
                    rec = rsb.tile([128, SLAB], F32, name="rec", tag="rec",
                                   bufs=2)
                    nc.vector.reciprocal_approx_fast(rec[:, :mw], bc[:, :mw])
                    for c in range(2):
                        o_sb = osb.tile([128, SLAB], F32, name="o_sb",
                                        tag="o", bufs=3)
                        nc.vector.tensor_mul(o_sb[:, :mw], oraw[c][:, :mw],
                                             rec[:, :mw])
                        nc.sync.dma_start(
                            d["out"][c * 128:(c + 1) * 128, m0:m0 + mw],
                            o_sb[:, :mw])

                pending_epi[0] = epilogue

            pending_epi[0]()


def build():
    nc = bacc.Bacc("TRN2", target_bir_lowering=False, debug=False,
                   enable_asserts=True, num_devices=N_CORES)
    d = {
        "x": nc.dram_tensor("x", [C, NPIX], F32, kind="ExternalInput").ap(),
        "wqT": nc.dram_tensor("wqT", [C, C], F32R, kind="ExternalInput").ap(),
        "wkT": nc.dram_tensor("wkT", [C, C], F32R, kind="ExternalInput").ap(),
        "wvT": nc.dram_tensor("wvT", [C, VTW], F32R,
                              kind="ExternalInput").ap(),
        "vbias": nc.dram_tensor("vbias", [128, VTW], F32,
                                kind="ExternalInput").ap(),
        "bq2": nc.dram_tensor("bq2", [128, 2], F32, kind="ExternalInput").ap(),
        "bk2": nc.dram_tensor("bk2", [128, 2], F32, kind="ExternalInput").ap(),
        "gnw2": nc.dram_tensor("gnw2", [128, 2], F32,
                               kind="ExternalInput").ap(),
        "gnb2": nc.dram_tensor("gnb2", [128, 2], F32,
                               kind="ExternalInput").ap(),
        "memb0": nc.dram_tensor("memb0", [128, 128], F32,
                                kind="ExternalInput").ap(),
        "memb1": nc.dram_tensor("memb1", [128, 128], F32,
                                kind="ExternalInput").ap(),
        "membT0": nc.dram_tensor("membT0", [G, 128], F32,
                                 kind="ExternalInput").ap(),
        "membT1": nc.dram_tensor("membT1", [G, 128], F32,
                                 kind="ExternalInput").ap(),
        "ones_r": nc.dram_tensor("ones_r", [1, 128], F32R,
                                 kind="ExternalInput").ap(),
        "onescol": nc.dram_tensor("onescol", [128, 128], F32R,
                                  kind="ExternalInput").ap(),
        "out": nc.dram_tensor("out", [C, NPIX], F32,
                              kind="ExternalOutput").ap(),
    }
    with tile.TileContext(nc) as tc:
        _emit(nc, tc, d)
    nc.compile()
    return nc


def _get_nc():
    if not _NC_CACHE:
        _NC_CACHE.append(build())
    return _NC_CACHE[0]


def _host_prep(input, Wq, bq, Wk, bk, Wv, bv, gn_w, gn_b):
    f32 = np.float32
    xs = np.ascontiguousarray(input.reshape(B, C, NPIX)).astype(f32, copy=False)
    wqT = np.ascontiguousarray(Wq.T).astype(f32, copy=False)
    wkT = np.ascontiguousarray(Wk.T).astype(f32, copy=False)
    wvT = np.ascontiguousarray(Wv.T).astype(f32, copy=False)
    vbias = np.ascontiguousarray(
        np.broadcast_to(bv.astype(f32), (128, VTW)))
    onescol = np.zeros((128, 128), f32)
    onescol[:, 0] = 1.0
    pairify = lambda v: np.ascontiguousarray(v.astype(f32).reshape(2, 128).T)
    p = np.arange(128)
    g = np.arange(128)
    memb0 = ((g[None, :] == p[:, None] // CPG) & (g[None, :] < G)).astype(f32)
    memb1 = ((g[None, :] == 16 + p[:, None] // CPG)
             & (g[None, :] < G)).astype(f32)
    shared = {
        "wqT": wqT, "wkT": wkT, "wvT": wvT, "vbias": vbias,
        "bq2": pairify(bq), "bk2": pairify(bk),
        "gnw2": pairify(gn_w), "gnb2": pairify(gn_b),
        "memb0": memb0, "memb1": memb1,
        "membT0": np.ascontiguousarray(memb0[:, :G].T),
        "membT1": np.ascontiguousarray(memb1[:, :G].T),
        "ones_r": np.ones((1, 128), f32),
        "onescol": onescol,
    }
    return [{"x": np.ascontiguousarray(xs[c]), **shared}
            for c in range(N_CORES)]


def run(inputs, trace=False):
    nc = _get_nc()
    in_maps = _host_prep(**inputs)
    res = run_bass_kernel_spmd(nc, in_maps, list(range(N_CORES)), trace=trace)
    out = np.stack([res.results[c]["out"] for c in range(N_CORES)])
    return out.reshape(B, C, HH, WW), res


def kernel(**inputs):
    out, _ = run(inputs, trace=False)
    return out


# revision 45
# speedup vs baseline: 1.2706x; 1.2706x over previous
"""AttentionBlock (GroupNorm -> QKV 1x1conv -> 2304x2304 spatial attention)
on 8 Trainium2 NeuronCores, data-parallel over batch.

Per core (one batch element b), with x = input[b] viewed as [C=256, N=2304]:
  gn    = groupnorm(x)                          (32 groups of 8 channels)
  q,k   = Wq@gn, Wk@gn        stored [C, N]     (channel on partitions)
  vT    = gn^T @ Wv^T (+ ones col)  [N, C+1]    (pixel on partitions)
  S^T   = k^T tiles . q       [n-tile, m]       (scores transposed)
  E     = exp(S^T / 16)                         (no max-sub: |scores| ~ 1)
  out   = (vT^T . E) / (ones^T . E)             (rowsum via vT's ones col)

All large matmuls run fp32r (full PE rate at free-dim >= 256).
"""
import numpy as np
from contextlib import ExitStack

import concourse.bass as bass
import concourse.tile as tile
from concourse import bacc, mybir
from concourse.bass_utils import run_bass_kernel_spmd

B, C, HH, WW = 8, 256, 48, 48
NPIX = HH * WW            # 2304
G = 32                    # groups
CPG = C // G              # 8 channels per group
EPS = 1e-5
N_CORES = 8
NT = NPIX // 128          # 18 pixel tiles
SLAB = 512
M_SLABS = [(j, min(SLAB, NPIX - j)) for j in range(0, NPIX, SLAB)]
VTW = C                   # vT slab width (no ones col; see onescol)
INV_GN = 1.0 / (CPG * NPIX)
QK_SCALE = 1.0 / 16.0     # 1/sqrt(C)

F32 = mybir.dt.float32
F32R = mybir.dt.float32r
AF = mybir.ActivationFunctionType
ALU = mybir.AluOpType
AX = mybir.AxisListType

_NC_CACHE = []


def _emit(nc, tc, d):
    with ExitStack() as ctx:
        pers = ctx.enter_context(tc.tile_pool(name="pers", bufs=1))

        # ---- persistent SBUF tensors ----
        x_t = [pers.tile([128, NPIX], F32, name=f"x{t}") for t in range(2)]
        xn_t = [pers.tile([128, NPIX], F32R, name=f"xn{t}") for t in range(2)]
        q_t = [pers.tile([128, NPIX], F32R, name=f"q{t}") for t in range(2)]
        k_t = [pers.tile([128, NPIX], F32R, name=f"k{t}") for t in range(2)]
        vt_all = pers.tile([128, NT * VTW], F32R, name="vt_all")

        wq = [pers.tile([128, C], F32R, name=f"wq{t}") for t in range(2)]
        wk = [pers.tile([128, C], F32R, name=f"wk{t}") for t in range(2)]
        wv = [pers.tile([128, VTW], F32R, name=f"wv{t}") for t in range(2)]
        vbias = pers.tile([128, VTW], F32, name="vbias")
        bq2 = pers.tile([128, 2], F32, name="bq2")
        bk2 = pers.tile([128, 2], F32, name="bk2")
        gnw2 = pers.tile([128, 2], F32, name="gnw2")
        gnb2 = pers.tile([128, 2], F32, name="gnb2")
        memb = [pers.tile([128, 128], F32, name=f"memb{t}") for t in range(2)]
        membT = [pers.tile([G, 128], F32, name=f"membT{t}") for t in range(2)]
        ones_r = pers.tile([1, 128], F32R, name="ones_r")
        onescol = pers.tile([128, 128], F32R, name="onescol")

        # Everything waits on GN stats, which need the full x: give x the
        # whole HBM bandwidth first (one tile per DMA queue), then the tiny
        # GN constants, then the projection weights (needed a few us later).
        HNP = NPIX // 2
        for h in range(2):
            nc.sync.dma_start(
                x_t[0][:, h * HNP:(h + 1) * HNP],
                d["x"][0:128, h * HNP:(h + 1) * HNP])
            nc.scalar.dma_start(
                x_t[1][:, h * HNP:(h + 1) * HNP],
                d["x"][128:256, h * HNP:(h + 1) * HNP])
        for t in range(2):
            nc.sync.dma_start(memb[t][:], d[f"memb{t}"][:])
            nc.scalar.dma_start(membT[t][:], d[f"membT{t}"][:])
        nc.sync.dma_start(gnw2[:], d["gnw2"][:])
        nc.scalar.dma_start(gnb2[:], d["gnb2"][:])
        for t in range(2):
            nc.sync.dma_start(wq[t][:], d["wqT"][t * 128:(t + 1) * 128, :])
            nc.scalar.dma_start(wk[t][:], d["wkT"][t * 128:(t + 1) * 128, :])
            nc.sync.dma_start(wv[t][:], d["wvT"][t * 128:(t + 1) * 128, :])
        nc.scalar.dma_start(vbias[:], d["vbias"][:])
        nc.sync.dma_start(bq2[:], d["bq2"][:])
        nc.scalar.dma_start(bk2[:], d["bk2"][:])
        nc.sync.dma_start(ones_r[:], d["ones_r"][:])
        nc.scalar.dma_start(onescol[:], d["onescol"][:])

        # ================= GroupNorm =================
        with ExitStack() as pctx:
            sc = pctx.enter_context(tc.tile_pool(name="gn_sc", bufs=2))
            gps = pctx.enter_context(
                tc.tile_pool(name="gn_ps", bufs=2, space="PSUM"))

            # per-partition mean/var in one DVE pass per half via bn_stats
            # (384-wide windows; 2304 = 6 x 384), aggregated by bn_aggr.
            # stats[t] = [mean_ch, E[x^2]_ch]
            BNW = 384
            stats = [sc.tile([128, 2], F32, name=f"stats{t}") for t in range(2)]
            for t in range(2):
                bnraw = sc.tile([128, 36], F32, name="bnraw", tag="bnr",
                                bufs=2)
                for w in range(6):
                    nc.vector.bn_stats(
                        bnraw[:, 6 * w:6 * w + 6],
                        x_t[t][:, w * BNW:(w + 1) * BNW])
                mv = sc.tile([128, 2], F32, name="mv", tag="mv", bufs=2)
                nc.vector.bn_aggr(mv[:], bnraw[:])
                m2 = sc.tile([128, 1], F32, name="m2", tag="m2", bufs=2)
                nc.vector.tensor_mul(m2[:], mv[:, 0:1], mv[:, 0:1])
                nc.vector.tensor_copy(stats[t][:, 0:1], mv[:, 0:1])
                nc.vector.tensor_add(stats[t][:, 1:2], mv[:, 1:2], m2[:])

            # group sums: partitions 0..31 of memb^T stats (memb cols 32..127
            # are zero padding -- fp32 matmul needs full col groups)
            g_ps = gps.tile([128, 2], F32, name="g_ps")
            for t in range(2):
                nc.tensor.matmul(g_ps[:], memb[t][:], stats[t][:],
                                 start=(t == 0), stop=(t == 1))

            mr = sc.tile([G, 2], F32, name="mr")          # [mean, rstd]
            tmp1 = sc.tile([G, 1], F32, name="tmp1")
            tmp2 = sc.tile([G, 1], F32, name="tmp2")
            tmp3 = sc.tile([G, 1], F32, name="tmp3")
            nc.vector.tensor_scalar_mul(mr[:, 0:1], g_ps[0:G, 0:1], 1.0 / CPG)
            nc.vector.tensor_scalar_mul(tmp1[:], g_ps[0:G, 1:2], 1.0 / CPG)
            nc.vector.tensor_mul(tmp2[:], mr[:, 0:1], mr[:, 0:1])
            nc.vector.tensor_sub(tmp3[:], tmp1[:], tmp2[:])       # var
            nc.vector.tensor_scalar_add(tmp3[:], tmp3[:], EPS)
            nc.scalar.activation(tmp1[:], tmp3[:], AF.Sqrt)
            nc.vector.reciprocal_approx_fast(mr[:, 1:2], tmp1[:])  # rstd

            # broadcast per-group (mean, rstd) back to channels; fold gn w/b
            ab = []
            for t in range(2):
                ch_ps = gps.tile([128, 2], F32, name="ch_ps", tag="ch", bufs=2)
                nc.tensor.matmul(ch_ps[:], membT[t][:], mr[:],
                                 start=True, stop=True)
                a_sb = sc.tile([128, 1], F32, name="a_sb", tag="a", bufs=2)
                t_sb = sc.tile([128, 1], F32, name="t_sb", tag="t", bufs=2)
                b_sb = sc.tile([128, 1], F32, name="b_sb", tag="b", bufs=2)
                nc.vector.tensor_mul(a_sb[:], ch_ps[:, 1:2], gnw2[:, t:t + 1])
                nc.vector.tensor_mul(t_sb[:], ch_ps[:, 0:1], a_sb[:])
                nc.vector.tensor_sub(b_sb[:], gnb2[:, t:t + 1], t_sb[:])
                ab.append((a_sb, b_sb))
            # normalize slab-major so slab-0 projections start early
            for (m0, mw) in M_SLABS:
                for t in range(2):
                    nc.vector.tensor_scalar(xn_t[t][:, m0:m0 + mw],
                                            x_t[t][:, m0:m0 + mw],
                                            ab[t][0][:], ab[t][1][:],
                                            op0=ALU.mult, op1=ALU.add)

        # ================= Q/K/V projections =================
        with ExitStack() as pctx:
            pps = pctx.enter_context(
                tc.tile_pool(name="proj_ps", bufs=2, space="PSUM"))

            for (m0, mw) in M_SLABS:
                for to in range(2):          # output-channel tile
                    for (w2, b2, dst) in ((wq, bq2, q_t), (wk, bk2, k_t)):
                        ps = pps.tile([128, SLAB], F32, name="qk_ps",
                                      tag="qk", bufs=2)
                        for tch in range(2):
                            nc.tensor.matmul(
                                ps[:, :mw],
                                w2[tch][:, to * 128:(to + 1) * 128],
                                xn_t[tch][:, m0:m0 + mw],
                                start=(tch == 0), stop=(tch == 1))
                        # ACT is idle here; keeps the DVE free for xn
                        # chunks and vT bias adds
                        nc.scalar.activation(
                            dst[to][:, m0:m0 + mw], ps[:, :mw],
                            AF.Identity, bias=b2[:, to:to + 1])

            for tn in range(NT):
                ps = pps.tile([128, VTW], F32, name="vt_ps", tag="vt", bufs=2)
                for tch in range(2):
                    nc.tensor.matmul(
                        ps[:],
                        xn_t[tch][:, tn * 128:(tn + 1) * 128],
                        wv[tch][:],
                        start=(tch == 0), stop=(tch == 1))
                nc.vector.tensor_add(
                    vt_all[:, tn * VTW:(tn + 1) * VTW], ps[:], vbias[:])

        # ================= attention main loop =================
        with ExitStack() as mctx:
            stp = mctx.enter_context(
                tc.tile_pool(name="st_ps", bufs=4, space="PSUM"))
            outp = mctx.enter_context(
                tc.tile_pool(name="out_ps", bufs=2, space="PSUM"))
            rowp = mctx.enter_context(
                tc.tile_pool(name="row_ps", bufs=1, space="PSUM"))
            bcp = mctx.enter_context(
                tc.tile_pool(name="bc_ps", bufs=1, space="PSUM"))
            epool = mctx.enter_context(tc.tile_pool(name="e_sb", bufs=4))
            osb = mctx.enter_context(tc.tile_pool(name="o_sb", bufs=3))
            orawp = mctx.enter_context(tc.tile_pool(name="oraw_sb", bufs=4))
            rsb = mctx.enter_context(tc.tile_pool(name="r_sb", bufs=2))

            PRE = 4  # S^T matmul prefill depth (= st_ps bufs)
            pending_epi = [None]

            for (m0, mw) in M_SLABS:
                st = {}

                def emit_s(t, m0=m0, mw=mw, st=st):
                    st[t] = stp.tile([128, SLAB], F32, name="st", tag="st",
                                     bufs=PRE)
                    for tch in range(2):
                        nc.tensor.matmul(
                            st[t][:, :mw],
                            k_t[tch][:, t * 128:(t + 1) * 128],
                            q_t[tch][:, m0:m0 + mw],
                            start=(tch == 0), stop=(tch == 1))

                oc = [outp.tile([128, SLAB], F32, name=f"oc{c}", tag="oc",
                                bufs=2) for c in range(2)]
                rows = rowp.tile([128, SLAB], F32, name="rows")

                for t in range(PRE):
                    emit_s(t)
                # previous slab's epilogue lands after this slab's S prefill
                # so the PE never sits on the denominator chain
                if pending_epi[0] is not None:
                    pending_epi[0]()
                    pending_epi[0] = None

                for t in range(NT):
                    e_t = epool.tile([128, SLAB], F32R, name="e_t", tag="e",
                                     bufs=4)
                    nc.scalar.activation(e_t[:, :mw], st[t][:, :mw], AF.Exp,
                                         scale=QK_SCALE)
                    st.pop(t)
                    if t + PRE < NT:
                        emit_s(t + PRE)
                    tn_off = t * VTW
                    for c in range(2):
                        nc.tensor.matmul(
                            oc[c][:, :mw],
                            vt_all[:, tn_off + c * 128:tn_off + (c + 1) * 128],
                            e_t[:, :mw],
                            start=(t == 0), stop=(t == NT - 1))
                    # rowsum: lhsT = [ones, 0, ..., 0] (fp32r needs M=128);
                    # the sum lands in output partition 0
                    nc.tensor.matmul(
                        rows[:, :mw],
                        onescol[:],
                        e_t[:, :mw],
                        start=(t == 0), stop=(t == NT - 1))

                # free PSUM banks immediately: raw sums -> SBUF
                oraw = []
                for c in range(2):
                    orw = orawp.tile([128, SLAB], F32, name="oraw",
                                     tag="oraw", bufs=4)
                    nc.vector.tensor_copy(orw[:, :mw], oc[c][:, :mw])
                    oraw.append(orw)
                rs_sb = rsb.tile([1, SLAB], F32R, name="rs_sb", tag="rs",
                                 bufs=2)
                nc.vector.tensor_copy(rs_sb[:, :mw], rows[0:1, :mw])

                def epilogue(oraw=oraw, rs_sb=rs_sb, m0=m0, mw=mw):
                    # denominators: broadcast rowsum to 128 partitions (K=1
                    # fp32 matmul), then fast reciprocal
                    bc = bcp.tile([128, SLAB], F32, name="bc")
                    nc.tensor.matmul(bc[:, :mw], ones_r[:], rs_sb[:, :mw],
                                     start=True, stop=True)
                    rec = rsb.tile([128, SLAB], F32, name="rec", tag="rec",
                                   bufs=2)
                    nc.vector.reciprocal_approx_fast(rec[:, :mw], bc[:, :mw])
                    for c in range(2):
                        o_sb = osb.tile([128, SLAB], F32, name="o_sb",
                                        tag="o", bufs=3)
                        nc.vector.tensor_mul(o_sb[:, :mw], oraw[c][:, :mw],
                                             rec[:, :mw])
                        nc.sync.dma_start(
                            d["out"][c * 128:(c + 1) * 128, m0:m0 + mw],
                            o_sb[:, :mw])

                pending_epi[0] = epilogue

            pending_epi[0]()


def build():
    nc = bacc.Bacc("TRN2", target_bir_lowering=False, debug=False,
                   enable_asserts=True, num_devices=N_CORES)
    d = {
        "x": nc.dram_tensor("x", [C, NPIX], F32, kind="ExternalInput").ap(),
        "wqT": nc.dram_tensor("wqT", [C, C], F32R, kind="ExternalInput").ap(),
        "wkT": nc.dram_tensor("wkT", [C, C], F32R, kind="ExternalInput").ap(),
        "wvT": nc.dram_tensor("wvT", [C, VTW], F32R,
                              kind="ExternalInput").ap(),
        "vbias": nc.dram_tensor("vbias", [128, VTW], F32,
                                kind="ExternalInput").ap(),
        "bq2": nc.dram_tensor("bq2", [128, 2], F32, kind="ExternalInput").ap(),
        "bk2": nc.dram_tensor("bk2", [128, 2], F32, kind="ExternalInput").ap(),
        "gnw2": nc.dram_tensor("gnw2", [128, 2], F32,
                               kind="ExternalInput").ap(),
        "gnb2": nc.dram_tensor("gnb2", [128, 2], F32,
                               kind="ExternalInput").ap(),
        "memb0": nc.dram_tensor("memb0", [128, 128], F32,
                                kind="ExternalInput").ap(),
        "memb1": nc.dram_tensor("memb1", [128, 128], F32,
                                kind="ExternalInput").ap(),
        "membT0": nc.dram_tensor("membT0", [G, 128], F32,
                                 kind="ExternalInput").ap(),
        "membT1": nc.dram_tensor("membT1", [G, 128], F32,
                                 kind="ExternalInput").ap(),
        "ones_r": nc.dram_tensor("ones_r", [1, 128], F32R,
                                 kind="ExternalInput").ap(),
        "onescol": nc.dram_tensor("onescol", [128, 128], F32R,
                                  kind="ExternalInput").ap(),
        "out": nc.dram_tensor("out", [C, NPIX], F32,
                              kind="ExternalOutput").ap(),
    }
    with tile.TileContext(nc) as tc:
        _emit(nc, tc, d)
    nc.compile()
    return nc


def _get_nc():
    if not _NC_CACHE:
        _NC_CACHE.append(build())
    return _NC_CACHE[0]


def _host_prep(input, Wq, bq, Wk, bk, Wv, bv, gn_w, gn_b):
    f32 = np.float32
    xs = np.ascontiguousarray(input.reshape(B, C, NPIX)).astype(f32, copy=False)
    wqT = np.ascontiguousarray(Wq.T).astype(f32, copy=False)
    wkT = np.ascontiguousarray(Wk.T).astype(f32, copy=False)
    wvT = np.ascontiguousarray(Wv.T).astype(f32, copy=False)
    vbias = np.ascontiguousarray(
        np.broadcast_to(bv.astype(f32), (128, VTW)))
    onescol = np.zeros((128, 128), f32)
    onescol[:, 0] = 1.0
    pairify = lambda v: np.ascontiguousarray(v.astype(f32).reshape(2, 128).T)
    p = np.arange(128)
    g = np.arange(128)
    memb0 = ((g[None, :] == p[:, None] // CPG) & (g[None, :] < G)).astype(f32)
    memb1 = ((g[None, :] == 16 + p[:, None] // CPG)
             & (g[None, :] < G)).astype(f32)
    shared = {
        "wqT": wqT, "wkT": wkT, "wvT": wvT, "vbias": vbias,
        "bq2": pairify(bq), "bk2": pairify(bk),
        "gnw2": pairify(gn_w), "gnb2": pairify(gn_b),
        "memb0": memb0, "memb1": memb1,
        "membT0": np.ascontiguousarray(memb0[:, :G].T),
        "membT1": np.ascontiguousarray(memb1[:, :G].T),
        "ones_r": np.ones((1, 128), f32),
        "onescol": onescol,
    }
    return [{"x": np.ascontiguousarray(xs[c]), **shared}
            for c in range(N_CORES)]


def run(inputs, trace=False):
    nc = _get_nc()
    in_maps = _host_prep(**inputs)
    res = run_bass_kernel_spmd(nc, in_maps, list(range(N_CORES)), trace=trace)
    out = np.stack([res.results[c]["out"] for c in range(N_CORES)])
    return out.reshape(B, C, HH, WW), res


def kernel(**inputs):
    out, _ = run(inputs, trace=False)
    return out


# revision 46
# speedup vs baseline: 1.3921x; 1.0956x over previous
"""AttentionBlock (GroupNorm -> QKV 1x1conv -> 2304x2304 spatial attention)
on 8 Trainium2 NeuronCores, data-parallel over batch.

Per core (one batch element b), with x = input[b] viewed as [C=256, N=2304]:
  gn    = groupnorm(x)                          (32 groups of 8 channels)
  q,k   = Wq@gn, Wk@gn        stored [C, N]     (channel on partitions)
  vT    = gn^T @ Wv^T (+ ones col)  [N, C+1]    (pixel on partitions)
  S^T   = k^T tiles . q       [n-tile, m]       (scores transposed)
  E     = exp(S^T / 16)                         (no max-sub: |scores| ~ 1)
  out   = (vT^T . E) / (ones^T . E)             (rowsum via vT's ones col)

All large matmuls run fp32r (full PE rate at free-dim >= 256).
"""
import numpy as np
from contextlib import ExitStack

import concourse.bass as bass
import concourse.tile as tile
from concourse import bacc, mybir
from concourse.bass_utils import run_bass_kernel_spmd

B, C, HH, WW = 8, 256, 48, 48
NPIX = HH * WW            # 2304
G = 32                    # groups
CPG = C // G              # 8 channels per group
EPS = 1e-5
N_CORES = 8
NT = NPIX // 128          # 18 pixel tiles
SLAB = 512
M_SLABS = [(j, min(SLAB, NPIX - j)) for j in range(0, NPIX, SLAB)]
VTW = C                   # vT slab width (no ones col; see onescol)
INV_GN = 1.0 / (CPG * NPIX)
QK_SCALE = 1.0 / 16.0     # 1/sqrt(C)

F32 = mybir.dt.float32
F32R = mybir.dt.float32r
AF = mybir.ActivationFunctionType
ALU = mybir.AluOpType
AX = mybir.AxisListType

_NC_CACHE = []


def _emit(nc, tc, d):
    with ExitStack() as ctx:
        pers = ctx.enter_context(tc.tile_pool(name="pers", bufs=1))

        # ---- persistent SBUF tensors ----
        x_t = [pers.tile([128, NPIX], F32, name=f"x{t}") for t in range(2)]
        xn_t = [pers.tile([128, NPIX], F32R, name=f"xn{t}") for t in range(2)]
        q_t = [pers.tile([128, NPIX], F32R, name=f"q{t}") for t in range(2)]
        k_t = [pers.tile([128, NPIX], F32R, name=f"k{t}") for t in range(2)]
        vt_all = pers.tile([128, NT * VTW], F32R, name="vt_all")

        wq = [pers.tile([128, C], F32R, name=f"wq{t}") for t in range(2)]
        wk = [pers.tile([128, C], F32R, name=f"wk{t}") for t in range(2)]
        wv = [pers.tile([128, VTW], F32R, name=f"wv{t}") for t in range(2)]
        vbias = pers.tile([128, VTW], F32, name="vbias")
        bq2 = pers.tile([128, 2], F32, name="bq2")
        bk2 = pers.tile([128, 2], F32, name="bk2")
        gnw2 = pers.tile([128, 2], F32, name="gnw2")
        gnb2 = pers.tile([128, 2], F32, name="gnb2")
        memb = [pers.tile([128, 128], F32, name=f"memb{t}") for t in range(2)]
        membT = [pers.tile([G, 128], F32, name=f"membT{t}") for t in range(2)]
        ones_r = pers.tile([1, 128], F32R, name="ones_r")
        onescol = pers.tile([128, 128], F32R, name="onescol")

        # Everything waits on GN stats, which need the full x: give x the
        # whole HBM bandwidth first (one tile per DMA queue), then the tiny
        # GN constants, then the projection weights (needed a few us later).
        HNP = NPIX // 2
        for h in range(2):
            nc.sync.dma_start(
                x_t[0][:, h * HNP:(h + 1) * HNP],
                d["x"][0:128, h * HNP:(h + 1) * HNP])
            nc.scalar.dma_start(
                x_t[1][:, h * HNP:(h + 1) * HNP],
                d["x"][128:256, h * HNP:(h + 1) * HNP])
        for t in range(2):
            nc.sync.dma_start(memb[t][:], d[f"memb{t}"][:])
            nc.scalar.dma_start(membT[t][:], d[f"membT{t}"][:])
        nc.sync.dma_start(gnw2[:], d["gnw2"][:])
        nc.scalar.dma_start(gnb2[:], d["gnb2"][:])
        for t in range(2):
            nc.sync.dma_start(wq[t][:], d["wqT"][t * 128:(t + 1) * 128, :])
            nc.scalar.dma_start(wk[t][:], d["wkT"][t * 128:(t + 1) * 128, :])
            nc.sync.dma_start(wv[t][:], d["wvT"][t * 128:(t + 1) * 128, :])
        nc.scalar.dma_start(vbias[:], d["vbias"][:])
        nc.sync.dma_start(bq2[:], d["bq2"][:])
        nc.scalar.dma_start(bk2[:], d["bk2"][:])
        nc.sync.dma_start(ones_r[:], d["ones_r"][:])
        nc.scalar.dma_start(onescol[:], d["onescol"][:])

        # ================= GroupNorm =================
        with ExitStack() as pctx:
            sc = pctx.enter_context(tc.tile_pool(name="gn_sc", bufs=2))
            gps = pctx.enter_context(
                tc.tile_pool(name="gn_ps", bufs=2, space="PSUM"))

            # per-partition mean/var in one DVE pass per half via bn_stats
            # (384-wide windows; 2304 = 6 x 384), aggregated by bn_aggr.
            # stats[t] = [mean_ch, E[x^2]_ch]
            BNW = 384
            stats = [sc.tile([128, 2], F32, name=f"stats{t}") for t in range(2)]
            for t in range(2):
                bnraw = sc.tile([128, 36], F32, name="bnraw", tag="bnr",
                                bufs=2)
                for w in range(6):
                    nc.vector.bn_stats(
                        bnraw[:, 6 * w:6 * w + 6],
                        x_t[t][:, w * BNW:(w + 1) * BNW])
                mv = sc.tile([128, 2], F32, name="mv", tag="mv", bufs=2)
                nc.vector.bn_aggr(mv[:], bnraw[:])
                m2 = sc.tile([128, 1], F32, name="m2", tag="m2", bufs=2)
                nc.vector.tensor_mul(m2[:], mv[:, 0:1], mv[:, 0:1])
                nc.vector.tensor_copy(stats[t][:, 0:1], mv[:, 0:1])
                nc.vector.tensor_add(stats[t][:, 1:2], mv[:, 1:2], m2[:])

            # group sums: partitions 0..31 of memb^T stats (memb cols 32..127
            # are zero padding -- fp32 matmul needs full col groups)
            g_ps = gps.tile([128, 2], F32, name="g_ps")
            for t in range(2):
                nc.tensor.matmul(g_ps[:], memb[t][:], stats[t][:],
                                 start=(t == 0), stop=(t == 1))

            mr = sc.tile([G, 2], F32, name="mr")          # [mean, rstd]
            tmp1 = sc.tile([G, 1], F32, name="tmp1")
            tmp2 = sc.tile([G, 1], F32, name="tmp2")
            tmp3 = sc.tile([G, 1], F32, name="tmp3")
            nc.vector.tensor_scalar_mul(mr[:, 0:1], g_ps[0:G, 0:1], 1.0 / CPG)
            nc.vector.tensor_scalar_mul(tmp1[:], g_ps[0:G, 1:2], 1.0 / CPG)
            nc.vector.tensor_mul(tmp2[:], mr[:, 0:1], mr[:, 0:1])
            nc.vector.tensor_sub(tmp3[:], tmp1[:], tmp2[:])       # var
            nc.vector.tensor_scalar_add(tmp3[:], tmp3[:], EPS)
            nc.scalar.activation(tmp1[:], tmp3[:], AF.Sqrt)
            nc.vector.reciprocal_approx_fast(mr[:, 1:2], tmp1[:])  # rstd

            # broadcast per-group (mean, rstd) back to channels; fold gn w/b
            ab = []
            for t in range(2):
                ch_ps = gps.tile([128, 2], F32, name="ch_ps", tag="ch", bufs=2)
                nc.tensor.matmul(ch_ps[:], membT[t][:], mr[:],
                                 start=True, stop=True)
                a_sb = sc.tile([128, 1], F32, name="a_sb", tag="a", bufs=2)
                t_sb = sc.tile([128, 1], F32, name="t_sb", tag="t", bufs=2)
                b_sb = sc.tile([128, 1], F32, name="b_sb", tag="b", bufs=2)
                nc.vector.tensor_mul(a_sb[:], ch_ps[:, 1:2], gnw2[:, t:t + 1])
                nc.vector.tensor_mul(t_sb[:], ch_ps[:, 0:1], a_sb[:])
                nc.vector.tensor_sub(b_sb[:], gnb2[:, t:t + 1], t_sb[:])
                ab.append((a_sb, b_sb))
            # normalize slab-major so slab-0 projections start early
            for (m0, mw) in M_SLABS:
                for t in range(2):
                    nc.vector.tensor_scalar(xn_t[t][:, m0:m0 + mw],
                                            x_t[t][:, m0:m0 + mw],
                                            ab[t][0][:], ab[t][1][:],
                                            op0=ALU.mult, op1=ALU.add)

        # ================= Q/K/V projections =================
        with ExitStack() as pctx:
            pps = pctx.enter_context(
                tc.tile_pool(name="proj_ps", bufs=2, space="PSUM"))

            for (m0, mw) in M_SLABS:
                for to in range(2):          # output-channel tile
                    for (w2, b2, dst) in ((wq, bq2, q_t), (wk, bk2, k_t)):
                        ps = pps.tile([128, SLAB], F32, name="qk_ps",
                                      tag="qk", bufs=2)
                        for tch in range(2):
                            nc.tensor.matmul(
                                ps[:, :mw],
                                w2[tch][:, to * 128:(to + 1) * 128],
                                xn_t[tch][:, m0:m0 + mw],
                                start=(tch == 0), stop=(tch == 1))
                        # ACT is idle here; keeps the DVE free for xn
                        # chunks and vT bias adds
                        nc.scalar.activation(
                            dst[to][:, m0:m0 + mw], ps[:, :mw],
                            AF.Identity, bias=b2[:, to:to + 1])

            for tn in range(NT):
                ps = pps.tile([128, VTW], F32, name="vt_ps", tag="vt", bufs=2)
                for tch in range(2):
                    nc.tensor.matmul(
                        ps[:],
                        xn_t[tch][:, tn * 128:(tn + 1) * 128],
                        wv[tch][:],
                        start=(tch == 0), stop=(tch == 1))
                nc.vector.tensor_add(
                    vt_all[:, tn * VTW:(tn + 1) * VTW], ps[:], vbias[:])

        # ================= attention main loop =================
        with ExitStack() as mctx:
            stp = mctx.enter_context(
                tc.tile_pool(name="st_ps", bufs=4, space="PSUM"))
            outp = mctx.enter_context(
                tc.tile_pool(name="out_ps", bufs=2, space="PSUM"))
            rowp = mctx.enter_context(
                tc.tile_pool(name="row_ps", bufs=1, space="PSUM"))
            bcp = mctx.enter_context(
                tc.tile_pool(name="bc_ps", bufs=1, space="PSUM"))
            epool = mctx.enter_context(tc.tile_pool(name="e_sb", bufs=4))
            osb = mctx.enter_context(tc.tile_pool(name="o_sb", bufs=3))
            orawp = mctx.enter_context(tc.tile_pool(name="oraw_sb", bufs=4))
            rsb = mctx.enter_context(tc.tile_pool(name="r_sb", bufs=2))

            PRE = 4  # S^T matmul prefill depth (= st_ps bufs)
            pending_epi = [None]

            for (m0, mw) in M_SLABS:
                st = {}

                def emit_s(t, m0=m0, mw=mw, st=st):
                    st[t] = stp.tile([128, SLAB], F32, name="st", tag="st",
                                     bufs=PRE)
                    for tch in range(2):
                        nc.tensor.matmul(
                            st[t][:, :mw],
                            k_t[tch][:, t * 128:(t + 1) * 128],
                            q_t[tch][:, m0:m0 + mw],
                            start=(tch == 0), stop=(tch == 1))

                oc = [outp.tile([128, SLAB], F32, name=f"oc{c}", tag="oc",
                                bufs=2) for c in range(2)]
                rows = rowp.tile([128, SLAB], F32, name="rows")

                for t in range(PRE):
                    emit_s(t)
                # previous slab's epilogue lands after this slab's S prefill
                # so the PE never sits on the denominator chain
                if pending_epi[0] is not None:
                    pending_epi[0]()
                    pending_epi[0] = None

                etri = []  # pending exp tiles for the 3-way rowsum pre-sum
                for t in range(NT):
                    e_t = epool.tile([128, SLAB], F32R, name="e_t", tag="e",
                                     bufs=6)
                    nc.scalar.activation(e_t[:, :mw], st[t][:, :mw], AF.Exp,
                                         scale=QK_SCALE)
                    st.pop(t)
                    if t + PRE < NT:
                        emit_s(t + PRE)
                    tn_off = t * VTW
                    for c in range(2):
                        nc.tensor.matmul(
                            oc[c][:, :mw],
                            vt_all[:, tn_off + c * 128:tn_off + (c + 1) * 128],
                            e_t[:, :mw],
                            start=(t == 0), stop=(t == NT - 1))
                    # denominators: pre-sum triples of exp tiles on the DVE,
                    # then one rowsum matmul per triple (lhsT = [ones,0,..,0],
                    # M=128 for fp32r; the sum lands in output partition 0)
                    etri.append(e_t)
                    if len(etri) == 3:
                        ea = epool.tile([128, SLAB], F32R, name="ea", tag="ea",
                                        bufs=2)
                        eb = epool.tile([128, SLAB], F32R, name="eb", tag="eb",
                                        bufs=2)
                        nc.vector.tensor_add(ea[:, :mw], etri[0][:, :mw],
                                             etri[1][:, :mw])
                        nc.vector.tensor_add(eb[:, :mw], ea[:, :mw],
                                             etri[2][:, :mw])
                        nc.tensor.matmul(
                            rows[:, :mw], onescol[:], eb[:, :mw],
                            start=(t == 2), stop=(t == NT - 1))
                        etri = []

                # free PSUM banks immediately: raw sums -> SBUF
                oraw = []
                for c in range(2):
                    orw = orawp.tile([128, SLAB], F32, name="oraw",
                                     tag="oraw", bufs=4)
                    nc.vector.tensor_copy(orw[:, :mw], oc[c][:, :mw])
                    oraw.append(orw)
                rs_sb = rsb.tile([1, SLAB], F32R, name="rs_sb", tag="rs",
                                 bufs=2)
                nc.vector.tensor_copy(rs_sb[:, :mw], rows[0:1, :mw])

                def epilogue(oraw=oraw, rs_sb=rs_sb, m0=m0, mw=mw):
                    # denominators: broadcast rowsum to 128 partitions (K=1
                    # fp32 matmul), then fast reciprocal
                    bc = bcp.tile([128, SLAB], F32, name="bc")
                    nc.tensor.matmul(bc[:, :mw], ones_r[:], rs_sb[:, :mw],
                                     start=True, stop=True)
                    rec = rsb.tile([128, SLAB], F32, name="rec", tag="rec",
                                   bufs=2)
                    nc.vector.reciprocal_approx_fast(rec[:, :mw], bc[:, :mw])
                    for c in range(2):
                        o_sb = osb.tile([128, SLAB], F32, name="o_sb",
                                        tag="o", bufs=3)
                        nc.vector.tensor_mul(o_sb[:, :mw], oraw[c][:, :mw],
                                             rec[:, :mw])
                        nc.sync.dma_start(
                            d["out"][c * 128:(c + 1) * 128, m0:m0 + mw],
                            o_sb[:, :mw])

                pending_epi[0] = epilogue

            pending_epi[0]()


def build():
    nc = bacc.Bacc("TRN2", target_bir_lowering=False, debug=False,
                   enable_asserts=True, num_devices=N_CORES)
    d = {
        "x": nc.dram_tensor("x", [C, NPIX], F32, kind="ExternalInput").ap(),
        "wqT": nc.dram_tensor("wqT", [C, C], F32R, kind="ExternalInput").ap(),
        "wkT": nc.dram_tensor("wkT", [C, C], F32R, kind="ExternalInput").ap(),
        "wvT": nc.dram_tensor("wvT", [C, VTW], F32R,
                              kind="ExternalInput").ap(),
        "vbias": nc.dram_tensor("vbias", [128, VTW], F32,
                                kind="ExternalInput").ap(),
        "bq2": nc.dram_tensor("bq2", [128, 2], F32, kind="ExternalInput").ap(),
        "bk2": nc.dram_tensor("bk2", [128, 2], F32, kind="ExternalInput").ap(),
        "gnw2": nc.dram_tensor("gnw2", [128, 2], F32,
                               kind="ExternalInput").ap(),
        "gnb2": nc.dram_tensor("gnb2", [128, 2], F32,
                               kind="ExternalInput").ap(),
        "memb0": nc.dram_tensor("memb0", [128, 128], F32,
                                kind="ExternalInput").ap(),
        "memb1": nc.dram_tensor("memb1", [128, 128], F32,
                                kind="ExternalInput").ap(),
        "membT0": nc.dram_tensor("membT0", [G, 128], F32,
                                 kind="ExternalInput").ap(),
        "membT1": nc.dram_tensor("membT1", [G, 128], F32,
                                 kind="ExternalInput").ap(),
        "ones_r": nc.dram_tensor("ones_r", [1, 128], F32R,
                                 kind="ExternalInput").ap(),
        "onescol": nc.dram_tensor("onescol", [128, 128], F32R,
                                  kind="ExternalInput").ap(),
        "out": nc.dram_tensor("out", [C, NPIX], F32,
                              kind="ExternalOutput").ap(),
    }
    with tile.TileContext(nc) as tc:
        _emit(nc, tc, d)
    nc.compile()
    return nc


def _get_nc():
    if not _NC_CACHE:
        _NC_CACHE.append(build())
    return _NC_CACHE[0]


def _host_prep(input, Wq, bq, Wk, bk, Wv, bv, gn_w, gn_b):
    f32 = np.float32
    xs = np.ascontiguousarray(input.reshape(B, C, NPIX)).astype(f32, copy=False)
    wqT = np.ascontiguousarray(Wq.T).astype(f32, copy=False)
    wkT = np.ascontiguousarray(Wk.T).astype(f32, copy=False)
    wvT = np.ascontiguousarray(Wv.T).astype(f32, copy=False)
    vbias = np.ascontiguousarray(
        np.broadcast_to(bv.astype(f32), (128, VTW)))
    onescol = np.zeros((128, 128), f32)
    onescol[:, 0] = 1.0
    pairify = lambda v: np.ascontiguousarray(v.astype(f32).reshape(2, 128).T)
    p = np.arange(128)
    g = np.arange(128)
    memb0 = ((g[None, :] == p[:, None] // CPG) & (g[None, :] < G)).astype(f32)
    memb1 = ((g[None, :] == 16 + p[:, None] // CPG)
             & (g[None, :] < G)).astype(f32)
    shared = {
        "wqT": wqT, "wkT": wkT, "wvT": wvT, "vbias": vbias,
        "bq2": pairify(bq), "bk2": pairify(bk),
        "gnw2": pairify(gn_w), "gnb2": pairify(gn_b),
        "memb0": memb0, "memb1": memb1,
        "membT0": np.ascontiguousarray(memb0[:, :G].T),
        "membT1": np.ascontiguousarray(memb1[:, :G].T),
        "ones_r": np.ones((1, 128), f32),
        "onescol": onescol,
    }
    return [{"x": np.ascontiguousarray(xs[c]), **shared}
            for c in range(N_CORES)]


def run(inputs, trace=False):
    nc = _get_nc()
    in_maps = _host_prep(**inputs)
    res = run_bass_kernel_spmd(nc, in_maps, list(range(N_CORES)), trace=trace)
    out = np.stack([res.results[c]["out"] for c in range(N_CORES)])
    return out.reshape(B, C, HH, WW), res


def kernel(**inputs):
    out, _ = run(inputs, trace=False)
    return out


# revision 48
# speedup vs baseline: 1.4149x; 1.0164x over previous
"""AttentionBlock (GroupNorm -> QKV 1x1conv -> 2304x2304 spatial attention)
on 8 Trainium2 NeuronCores, data-parallel over batch.

Per core (one batch element b), with x = input[b] viewed as [C=256, N=2304]:
  gn    = groupnorm(x)                          (32 groups of 8 channels)
  q,k   = Wq@gn, Wk@gn        stored [C, N]     (channel on partitions)
  vT    = gn^T @ Wv^T (+ ones col)  [N, C+1]    (pixel on partitions)
  S^T   = k^T tiles . q       [n-tile, m]       (scores transposed)
  E     = exp(S^T / 16)                         (no max-sub: |scores| ~ 1)
  out   = (vT^T . E) / (ones^T . E)             (rowsum via vT's ones col)

All large matmuls run fp32r (full PE rate at free-dim >= 256).
"""
import numpy as np
from contextlib import ExitStack

import concourse.bass as bass
import concourse.tile as tile
from concourse import bacc, mybir
from concourse.bass_utils import run_bass_kernel_spmd

B, C, HH, WW = 8, 256, 48, 48
NPIX = HH * WW            # 2304
G = 32                    # groups
CPG = C // G              # 8 channels per group
EPS = 1e-5
N_CORES = 8
NT = NPIX // 128          # 18 pixel tiles
SLAB = 512
M_SLABS = [(j, min(SLAB, NPIX - j)) for j in range(0, NPIX, SLAB)]
VTW = C                   # vT slab width (no ones col; see onescol)
INV_GN = 1.0 / (CPG * NPIX)
QK_SCALE = 1.0 / 16.0     # 1/sqrt(C)

F32 = mybir.dt.float32
F32R = mybir.dt.float32r
AF = mybir.ActivationFunctionType
ALU = mybir.AluOpType
AX = mybir.AxisListType

_NC_CACHE = []


def _emit(nc, tc, d):
    with ExitStack() as ctx:
        pers = ctx.enter_context(tc.tile_pool(name="pers", bufs=1))

        # ---- persistent SBUF tensors ----
        x_t = [pers.tile([128, NPIX], F32, name=f"x{t}") for t in range(2)]
        xn_t = [pers.tile([128, NPIX], F32R, name=f"xn{t}") for t in range(2)]
        q_t = [pers.tile([128, NPIX], F32R, name=f"q{t}") for t in range(2)]
        k_t = [pers.tile([128, NPIX], F32R, name=f"k{t}") for t in range(2)]
        vt_all = pers.tile([128, NT * VTW], F32R, name="vt_all")

        wq = [pers.tile([128, C], F32R, name=f"wq{t}") for t in range(2)]
        wk = [pers.tile([128, C], F32R, name=f"wk{t}") for t in range(2)]
        wv = [pers.tile([128, VTW], F32R, name=f"wv{t}") for t in range(2)]
        vbias = pers.tile([128, VTW], F32, name="vbias")
        bq2 = pers.tile([128, 2], F32, name="bq2")
        bk2 = pers.tile([128, 2], F32, name="bk2")
        gnw2 = pers.tile([128, 2], F32, name="gnw2")
        gnb2 = pers.tile([128, 2], F32, name="gnb2")
        memb = [pers.tile([128, 128], F32, name=f"memb{t}") for t in range(2)]
        membT = [pers.tile([G, 128], F32, name=f"membT{t}") for t in range(2)]
        ones_r = pers.tile([1, 128], F32R, name="ones_r")
        onescol = pers.tile([128, 128], F32R, name="onescol")

        # Everything waits on GN stats, which need the full x: give x the
        # whole HBM bandwidth first (one tile per DMA queue), then the tiny
        # GN constants, then the projection weights (needed a few us later).
        HNP = NPIX // 2
        for h in range(2):
            nc.sync.dma_start(
                x_t[0][:, h * HNP:(h + 1) * HNP],
                d["x"][0:128, h * HNP:(h + 1) * HNP])
            nc.scalar.dma_start(
                x_t[1][:, h * HNP:(h + 1) * HNP],
                d["x"][128:256, h * HNP:(h + 1) * HNP])
        for t in range(2):
            nc.sync.dma_start(memb[t][:], d[f"memb{t}"][:])
            nc.scalar.dma_start(membT[t][:], d[f"membT{t}"][:])
        nc.sync.dma_start(gnw2[:], d["gnw2"][:])
        nc.scalar.dma_start(gnb2[:], d["gnb2"][:])
        for t in range(2):
            nc.sync.dma_start(wq[t][:], d["wqT"][t * 128:(t + 1) * 128, :])
            nc.scalar.dma_start(wk[t][:], d["wkT"][t * 128:(t + 1) * 128, :])
            nc.sync.dma_start(wv[t][:], d["wvT"][t * 128:(t + 1) * 128, :])
        nc.scalar.dma_start(vbias[:], d["vbias"][:])
        nc.sync.dma_start(bq2[:], d["bq2"][:])
        nc.scalar.dma_start(bk2[:], d["bk2"][:])
        nc.sync.dma_start(ones_r[:], d["ones_r"][:])
        nc.scalar.dma_start(onescol[:], d["onescol"][:])

        # ================= GroupNorm =================
        with ExitStack() as pctx:
            sc = pctx.enter_context(tc.tile_pool(name="gn_sc", bufs=2))
            gps = pctx.enter_context(
                tc.tile_pool(name="gn_ps", bufs=2, space="PSUM"))

            # per-partition mean/var in one DVE pass per half via bn_stats
            # (384-wide windows; 2304 = 6 x 384), aggregated by bn_aggr.
            # stats[t] = [mean_ch, E[x^2]_ch]
            BNW = 384
            stats = [sc.tile([128, 2], F32, name=f"stats{t}") for t in range(2)]
            for t in range(2):
                bnraw = sc.tile([128, 36], F32, name="bnraw", tag="bnr",
                                bufs=2)
                for w in range(6):
                    nc.vector.bn_stats(
                        bnraw[:, 6 * w:6 * w + 6],
                        x_t[t][:, w * BNW:(w + 1) * BNW])
                mv = sc.tile([128, 2], F32, name="mv", tag="mv", bufs=2)
                nc.vector.bn_aggr(mv[:], bnraw[:])
                m2 = sc.tile([128, 1], F32, name="m2", tag="m2", bufs=2)
                nc.vector.tensor_mul(m2[:], mv[:, 0:1], mv[:, 0:1])
                nc.vector.tensor_copy(stats[t][:, 0:1], mv[:, 0:1])
                nc.vector.tensor_add(stats[t][:, 1:2], mv[:, 1:2], m2[:])

            # group sums: partitions 0..31 of memb^T stats (memb cols 32..127
            # are zero padding -- fp32 matmul needs full col groups)
            g_ps = gps.tile([128, 2], F32, name="g_ps")
            for t in range(2):
                nc.tensor.matmul(g_ps[:], memb[t][:], stats[t][:],
                                 start=(t == 0), stop=(t == 1))

            mr = sc.tile([G, 2], F32, name="mr")          # [mean, rstd]
            tmp1 = sc.tile([G, 1], F32, name="tmp1")
            tmp2 = sc.tile([G, 1], F32, name="tmp2")
            tmp3 = sc.tile([G, 1], F32, name="tmp3")
            nc.vector.tensor_scalar_mul(mr[:, 0:1], g_ps[0:G, 0:1], 1.0 / CPG)
            nc.vector.tensor_scalar_mul(tmp1[:], g_ps[0:G, 1:2], 1.0 / CPG)
            nc.vector.tensor_mul(tmp2[:], mr[:, 0:1], mr[:, 0:1])
            nc.vector.tensor_sub(tmp3[:], tmp1[:], tmp2[:])       # var
            nc.vector.tensor_scalar_add(tmp3[:], tmp3[:], EPS)
            nc.scalar.activation(tmp1[:], tmp3[:], AF.Sqrt)
            nc.vector.reciprocal_approx_fast(mr[:, 1:2], tmp1[:])  # rstd

            # broadcast per-group (mean, rstd) back to channels; fold gn w/b
            ab = []
            for t in range(2):
                ch_ps = gps.tile([128, 2], F32, name="ch_ps", tag="ch", bufs=2)
                nc.tensor.matmul(ch_ps[:], membT[t][:], mr[:],
                                 start=True, stop=True)
                a_sb = sc.tile([128, 1], F32, name="a_sb", tag="a", bufs=2)
                t_sb = sc.tile([128, 1], F32, name="t_sb", tag="t", bufs=2)
                b_sb = sc.tile([128, 1], F32, name="b_sb", tag="b", bufs=2)
                nc.vector.tensor_mul(a_sb[:], ch_ps[:, 1:2], gnw2[:, t:t + 1])
                nc.vector.tensor_mul(t_sb[:], ch_ps[:, 0:1], a_sb[:])
                nc.vector.tensor_sub(b_sb[:], gnb2[:, t:t + 1], t_sb[:])
                ab.append((a_sb, b_sb))
            # normalize slab-major so slab-0 projections start early
            for (m0, mw) in M_SLABS:
                for t in range(2):
                    nc.vector.tensor_scalar(xn_t[t][:, m0:m0 + mw],
                                            x_t[t][:, m0:m0 + mw],
                                            ab[t][0][:], ab[t][1][:],
                                            op0=ALU.mult, op1=ALU.add)

        # ================= Q/K/V projections =================
        with ExitStack() as pctx:
            pps = pctx.enter_context(
                tc.tile_pool(name="proj_ps", bufs=2, space="PSUM"))

            for (m0, mw) in M_SLABS:
                for to in range(2):          # output-channel tile
                    for (w2, b2, dst) in ((wq, bq2, q_t), (wk, bk2, k_t)):
                        ps = pps.tile([128, SLAB], F32, name="qk_ps",
                                      tag="qk", bufs=2)
                        for tch in range(2):
                            nc.tensor.matmul(
                                ps[:, :mw],
                                w2[tch][:, to * 128:(to + 1) * 128],
                                xn_t[tch][:, m0:m0 + mw],
                                start=(tch == 0), stop=(tch == 1))
                        # ACT is idle here; keeps the DVE free for xn
                        # chunks and vT bias adds
                        nc.scalar.activation(
                            dst[to][:, m0:m0 + mw], ps[:, :mw],
                            AF.Identity, bias=b2[:, to:to + 1])

            for tn in range(NT):
                ps = pps.tile([128, VTW], F32, name="vt_ps", tag="vt", bufs=2)
                for tch in range(2):
                    nc.tensor.matmul(
                        ps[:],
                        xn_t[tch][:, tn * 128:(tn + 1) * 128],
                        wv[tch][:],
                        start=(tch == 0), stop=(tch == 1))
                nc.vector.tensor_add(
                    vt_all[:, tn * VTW:(tn + 1) * VTW], ps[:], vbias[:])

        # ================= attention main loop =================
        with ExitStack() as mctx:
            stp = mctx.enter_context(
                tc.tile_pool(name="st_ps", bufs=4, space="PSUM"))
            outp = mctx.enter_context(
                tc.tile_pool(name="out_ps", bufs=2, space="PSUM"))
            rowp = mctx.enter_context(
                tc.tile_pool(name="row_ps", bufs=1, space="PSUM"))
            bcp = mctx.enter_context(
                tc.tile_pool(name="bc_ps", bufs=1, space="PSUM"))
            epool = mctx.enter_context(tc.tile_pool(name="e_sb", bufs=4))
            osb = mctx.enter_context(tc.tile_pool(name="o_sb", bufs=3))
            orawp = mctx.enter_context(tc.tile_pool(name="oraw_sb", bufs=4))
            rsb = mctx.enter_context(tc.tile_pool(name="r_sb", bufs=2))

            PRE = 4  # S^T matmul prefill depth (= st_ps bufs)
            pending_epi = [None]

            for (m0, mw) in M_SLABS:
                st = {}

                def emit_s(t, m0=m0, mw=mw, st=st):
                    st[t] = stp.tile([128, SLAB], F32, name="st", tag="st",
                                     bufs=PRE)
                    for tch in range(2):
                        nc.tensor.matmul(
                            st[t][:, :mw],
                            k_t[tch][:, t * 128:(t + 1) * 128],
                            q_t[tch][:, m0:m0 + mw],
                            start=(tch == 0), stop=(tch == 1))

                oc = [outp.tile([128, SLAB], F32, name=f"oc{c}", tag="oc",
                                bufs=2) for c in range(2)]
                rows = rowp.tile([128, SLAB], F32, name="rows")

                for t in range(PRE):
                    emit_s(t)
                # previous slab's epilogue lands after this slab's S prefill
                # so the PE never sits on the denominator chain
                if pending_epi[0] is not None:
                    pending_epi[0]()
                    pending_epi[0] = None

                etri = []  # pending exp tiles for the 3-way rowsum pre-sum
                for t in range(NT):
                    e_t = epool.tile([128, SLAB], F32R, name="e_t", tag="e",
                                     bufs=6)
                    nc.scalar.activation(e_t[:, :mw], st[t][:, :mw], AF.Exp,
                                         scale=QK_SCALE)
                    st.pop(t)
                    if t + PRE < NT:
                        emit_s(t + PRE)
                    tn_off = t * VTW
                    for c in range(2):
                        nc.tensor.matmul(
                            oc[c][:, :mw],
                            vt_all[:, tn_off + c * 128:tn_off + (c + 1) * 128],
                            e_t[:, :mw],
                            start=(t == 0), stop=(t == NT - 1))
                    # denominators: pre-sum triples of exp tiles on the DVE,
                    # then one rowsum matmul per triple (lhsT = [ones,0,..,0],
                    # M=128 for fp32r; the sum lands in output partition 0)
                    etri.append(e_t)
                    if len(etri) == 6:
                        acc = None
                        for j in range(1, 6):
                            nxt = epool.tile([128, SLAB], F32R, name="ea",
                                             tag=f"ea{j % 2}", bufs=2)
                            nc.vector.tensor_add(
                                nxt[:, :mw],
                                (etri[0] if acc is None else acc)[:, :mw],
                                etri[j][:, :mw])
                            acc = nxt
                        nc.tensor.matmul(
                            rows[:, :mw], onescol[:], acc[:, :mw],
                            start=(t == 5), stop=(t == NT - 1))
                        etri = []

                # free PSUM banks immediately: raw sums -> SBUF (on ACT --
                # Exp and Identity tables both stay resident)
                oraw = []
                for c in range(2):
                    orw = orawp.tile([128, SLAB], F32, name="oraw",
                                     tag="oraw", bufs=4)
                    nc.scalar.activation(orw[:, :mw], oc[c][:, :mw],
                                         AF.Identity)
                    oraw.append(orw)
                rs_sb = rsb.tile([1, SLAB], F32R, name="rs_sb", tag="rs",
                                 bufs=2)
                nc.vector.tensor_copy(rs_sb[:, :mw], rows[0:1, :mw])

                def epilogue(oraw=oraw, rs_sb=rs_sb, m0=m0, mw=mw):
                    # denominators: broadcast rowsum to 128 partitions (K=1
                    # fp32 matmul), then fast reciprocal
                    bc = bcp.tile([128, SLAB], F32, name="bc")
                    nc.tensor.matmul(bc[:, :mw], ones_r[:], rs_sb[:, :mw],
                                     start=True, stop=True)
                    rec = rsb.tile([128, SLAB], F32, name="rec", tag="rec",
                                   bufs=2)
                    nc.vector.reciprocal_approx_fast(rec[:, :mw], bc[:, :mw])
                    for c in range(2):
                        o_sb = osb.tile([128, SLAB], F32, name="o_sb",
                                        tag="o", bufs=3)
                        nc.vector.tensor_mul(o_sb[:, :mw], oraw[c][:, :mw],
                                             rec[:, :mw])
                        nc.sync.dma_start(
                            d["out"][c * 128:(c + 1) * 128, m0:m0 + mw],
                            o_sb[:, :mw])

                pending_epi[0] = epilogue

            pending_epi[0]()


def build():
    nc = bacc.Bacc("TRN2", target_bir_lowering=False, debug=False,
                   enable_asserts=True, num_devices=N_CORES)
    d = {
        "x": nc.dram_tensor("x", [C, NPIX], F32, kind="ExternalInput").ap(),
        "wqT": nc.dram_tensor("wqT", [C, C], F32R, kind="ExternalInput").ap(),
        "wkT": nc.dram_tensor("wkT", [C, C], F32R, kind="ExternalInput").ap(),
        "wvT": nc.dram_tensor("wvT", [C, VTW], F32R,
                              kind="ExternalInput").ap(),
        "vbias": nc.dram_tensor("vbias", [128, VTW], F32,
                                kind="ExternalInput").ap(),
        "bq2": nc.dram_tensor("bq2", [128, 2], F32, kind="ExternalInput").ap(),
        "bk2": nc.dram_tensor("bk2", [128, 2], F32, kind="ExternalInput").ap(),
        "gnw2": nc.dram_tensor("gnw2", [128, 2], F32,
                               kind="ExternalInput").ap(),
        "gnb2": nc.dram_tensor("gnb2", [128, 2], F32,
                               kind="ExternalInput").ap(),
        "memb0": nc.dram_tensor("memb0", [128, 128], F32,
                                kind="ExternalInput").ap(),
        "memb1": nc.dram_tensor("memb1", [128, 128], F32,
                                kind="ExternalInput").ap(),
        "membT0": nc.dram_tensor("membT0", [G, 128], F32,
                                 kind="ExternalInput").ap(),
        "membT1": nc.dram_tensor("membT1", [G, 128], F32,
                                 kind="ExternalInput").ap(),
        "ones_r": nc.dram_tensor("ones_r", [1, 128], F32R,
                                 kind="ExternalInput").ap(),
        "onescol": nc.dram_tensor("onescol", [128, 128], F32R,
                                  kind="ExternalInput").ap(),
        "out": nc.dram_tensor("out", [C, NPIX], F32,
                              kind="ExternalOutput").ap(),
    }
    with tile.TileContext(nc) as tc:
        _emit(nc, tc, d)
    nc.compile()
    return nc


def _get_nc():
    if not _NC_CACHE:
        _NC_CACHE.append(build())
    return _NC_CACHE[0]


def _host_prep(input, Wq, bq, Wk, bk, Wv, bv, gn_w, gn_b):
    f32 = np.float32
    xs = np.ascontiguousarray(input.reshape(B, C, NPIX)).astype(f32, copy=False)
    wqT = np.ascontiguousarray(Wq.T).astype(f32, copy=False)
    wkT = np.ascontiguousarray(Wk.T).astype(f32, copy=False)
    wvT = np.ascontiguousarray(Wv.T).astype(f32, copy=False)
    vbias = np.ascontiguousarray(
        np.broadcast_to(bv.astype(f32), (128, VTW)))
    onescol = np.zeros((128, 128), f32)
    onescol[:, 0] = 1.0
    pairify = lambda v: np.ascontiguousarray(v.astype(f32).reshape(2, 128).T)
    p = np.arange(128)
    g = np.arange(128)
    memb0 = ((g[None, :] == p[:, None] // CPG) & (g[None, :] < G)).astype(f32)
    memb1 = ((g[None, :] == 16 + p[:, None] // CPG)
             & (g[None, :] < G)).astype(f32)
    shared = {
        "wqT": wqT, "wkT": wkT, "wvT": wvT, "vbias": vbias,
        "bq2": pairify(bq), "bk2": pairify(bk),
        "gnw2": pairify(gn_w), "gnb2": pairify(gn_b),
        "memb0": memb0, "memb1": memb1,
        "membT0": np.ascontiguousarray(memb0[:, :G].T),
        "membT1": np.ascontiguousarray(memb1[:, :G].T),
        "ones_r": np.ones((1, 128), f32),
        "onescol": onescol,
    }
    return [{"x": np.ascontiguousarray(xs[c]), **shared}
            for c in range(N_CORES)]


def run(inputs, trace=False):
    nc = _get_nc()
    in_maps = _host_prep(**inputs)
    res = run_bass_kernel_spmd(nc, in_maps, list(range(N_CORES)), trace=trace)
    out = np.stack([res.results[c]["out"] for c in range(N_CORES)])
    return out.reshape(B, C, HH, WW), res


def kernel(**inputs):
    out, _ = run(inputs, trace=False)
    return out


# revision 57
# speedup vs baseline: 1.4563x; 1.0293x over previous
"""AttentionBlock (GroupNorm -> QKV 1x1conv -> 2304x2304 spatial attention)
on 8 Trainium2 NeuronCores, data-parallel over batch.

Per core (one batch element b), with x = input[b] viewed as [C=256, N=2304]:
  gn    = groupnorm(x)                          (32 groups of 8 channels)
  q,k   = Wq@gn, Wk@gn        stored [C, N]     (channel on partitions)
  vT    = gn^T @ Wv^T (+ ones col)  [N, C+1]    (pixel on partitions)
  S^T   = k^T tiles . q       [n-tile, m]       (scores transposed)
  E     = exp(S^T / 16)                         (no max-sub: |scores| ~ 1)
  out   = (vT^T . E) / (ones^T . E)             (rowsum via vT's ones col)

All large matmuls run fp32r (full PE rate at free-dim >= 256).
"""
import numpy as np
from contextlib import ExitStack

import concourse.bass as bass
import concourse.tile as tile
from concourse import bacc, mybir
from concourse.bass_utils import run_bass_kernel_spmd

B, C, HH, WW = 8, 256, 48, 48
NPIX = HH * WW            # 2304
G = 32                    # groups
CPG = C // G              # 8 channels per group
EPS = 1e-5
N_CORES = 8
NT = NPIX // 128          # 18 pixel tiles
SLAB = 512
M_SLABS = [(j, min(SLAB, NPIX - j)) for j in range(0, NPIX, SLAB)]
VTW = C                   # vT slab width (no ones col; see onescol)
INV_GN = 1.0 / (CPG * NPIX)
QK_SCALE = 1.0 / 16.0     # 1/sqrt(C)

F32 = mybir.dt.float32
F32R = mybir.dt.float32r
AF = mybir.ActivationFunctionType
ALU = mybir.AluOpType
AX = mybir.AxisListType

_NC_CACHE = []


def _emit(nc, tc, d):
    with ExitStack() as ctx:
        pers = ctx.enter_context(tc.tile_pool(name="pers", bufs=1))

        # ---- persistent SBUF tensors ----
        x_t = [pers.tile([128, NPIX], F32, name=f"x{t}") for t in range(2)]
        xn_t = [pers.tile([128, NPIX], F32R, name=f"xn{t}") for t in range(2)]
        q_t = [pers.tile([128, NPIX], F32R, name=f"q{t}") for t in range(2)]
        k_t = [pers.tile([128, NPIX], F32R, name=f"k{t}") for t in range(2)]
        vt_all = pers.tile([128, NT * VTW], F32R, name="vt_all")

        wq = [pers.tile([128, C], F32R, name=f"wq{t}") for t in range(2)]
        wk = [pers.tile([128, C], F32R, name=f"wk{t}") for t in range(2)]
        wv = [pers.tile([128, VTW], F32R, name=f"wv{t}") for t in range(2)]
        vbias = pers.tile([128, VTW], F32, name="vbias")
        bq2 = pers.tile([128, 2], F32, name="bq2")
        bk2 = pers.tile([128, 2], F32, name="bk2")
        gnw2 = pers.tile([128, 2], F32, name="gnw2")
        gnb2 = pers.tile([128, 2], F32, name="gnb2")
        memb = [pers.tile([128, 128], F32, name=f"memb{t}") for t in range(2)]
        membT = [pers.tile([G, 128], F32, name=f"membT{t}") for t in range(2)]
        onescol = pers.tile([128, 128], F32R, name="onescol")

        # Everything waits on GN stats, which need the full x: give x the
        # whole HBM bandwidth first (one tile per DMA queue), then the tiny
        # GN constants, then the projection weights (needed a few us later).
        HNP = NPIX // 2
        for h in range(2):
            nc.sync.dma_start(
                x_t[0][:, h * HNP:(h + 1) * HNP],
                d["x"][0:128, h * HNP:(h + 1) * HNP])
            nc.scalar.dma_start(
                x_t[1][:, h * HNP:(h + 1) * HNP],
                d["x"][128:256, h * HNP:(h + 1) * HNP])
        for t in range(2):
            nc.sync.dma_start(memb[t][:], d[f"memb{t}"][:])
            nc.scalar.dma_start(membT[t][:], d[f"membT{t}"][:])
        nc.sync.dma_start(gnw2[:], d["gnw2"][:])
        nc.scalar.dma_start(gnb2[:], d["gnb2"][:])
        for t in range(2):
            nc.sync.dma_start(wq[t][:], d["wqT"][t * 128:(t + 1) * 128, :])
            nc.scalar.dma_start(wk[t][:], d["wkT"][t * 128:(t + 1) * 128, :])
            nc.sync.dma_start(wv[t][:], d["wvT"][t * 128:(t + 1) * 128, :])
        nc.scalar.dma_start(vbias[:], d["vbias"][:])
        nc.sync.dma_start(bq2[:], d["bq2"][:])
        nc.scalar.dma_start(bk2[:], d["bk2"][:])
        nc.scalar.dma_start(onescol[:], d["onescol"][:])

        # ================= GroupNorm =================
        with ExitStack() as pctx:
            sc = pctx.enter_context(tc.tile_pool(name="gn_sc", bufs=2))
            gps = pctx.enter_context(
                tc.tile_pool(name="gn_ps", bufs=2, space="PSUM"))

            # per-partition mean/var in one DVE pass per half via bn_stats
            # (384-wide windows; 2304 = 6 x 384), aggregated by bn_aggr.
            # stats[t] = [mean_ch, E[x^2]_ch]
            BNW = 384
            stats = [sc.tile([128, 2], F32, name=f"stats{t}") for t in range(2)]
            for t in range(2):
                bnraw = sc.tile([128, 36], F32, name="bnraw", tag="bnr",
                                bufs=2)
                for w in range(6):
                    nc.vector.bn_stats(
                        bnraw[:, 6 * w:6 * w + 6],
                        x_t[t][:, w * BNW:(w + 1) * BNW])
                mv = sc.tile([128, 2], F32, name="mv", tag="mv", bufs=2)
                nc.vector.bn_aggr(mv[:], bnraw[:])
                m2 = sc.tile([128, 1], F32, name="m2", tag="m2", bufs=2)
                nc.vector.tensor_mul(m2[:], mv[:, 0:1], mv[:, 0:1])
                nc.vector.tensor_copy(stats[t][:, 0:1], mv[:, 0:1])
                nc.vector.tensor_add(stats[t][:, 1:2], mv[:, 1:2], m2[:])

            # group sums: partitions 0..31 of memb^T stats (memb cols 32..127
            # are zero padding -- fp32 matmul needs full col groups)
            g_ps = gps.tile([128, 2], F32, name="g_ps")
            for t in range(2):
                nc.tensor.matmul(g_ps[:], memb[t][:], stats[t][:],
                                 start=(t == 0), stop=(t == 1))

            mr = sc.tile([G, 2], F32, name="mr")          # [mean, rstd]
            tmp1 = sc.tile([G, 1], F32, name="tmp1")
            tmp2 = sc.tile([G, 1], F32, name="tmp2")
            tmp3 = sc.tile([G, 1], F32, name="tmp3")
            nc.vector.tensor_scalar_mul(mr[:, 0:1], g_ps[0:G, 0:1], 1.0 / CPG)
            nc.vector.tensor_scalar_mul(tmp1[:], g_ps[0:G, 1:2], 1.0 / CPG)
            nc.vector.tensor_mul(tmp2[:], mr[:, 0:1], mr[:, 0:1])
            nc.vector.tensor_sub(tmp3[:], tmp1[:], tmp2[:])       # var
            nc.vector.tensor_scalar_add(tmp3[:], tmp3[:], EPS)
            nc.scalar.activation(tmp1[:], tmp3[:], AF.Sqrt)
            nc.vector.reciprocal_approx_fast(mr[:, 1:2], tmp1[:])  # rstd

            # broadcast per-group (mean, rstd) back to channels; fold gn w/b
            ab = []
            for t in range(2):
                ch_ps = gps.tile([128, 2], F32, name="ch_ps", tag="ch", bufs=2)
                nc.tensor.matmul(ch_ps[:], membT[t][:], mr[:],
                                 start=True, stop=True)
                a_sb = sc.tile([128, 1], F32, name="a_sb", tag="a", bufs=2)
                t_sb = sc.tile([128, 1], F32, name="t_sb", tag="t", bufs=2)
                b_sb = sc.tile([128, 1], F32, name="b_sb", tag="b", bufs=2)
                nc.vector.tensor_mul(a_sb[:], ch_ps[:, 1:2], gnw2[:, t:t + 1])
                nc.vector.tensor_mul(t_sb[:], ch_ps[:, 0:1], a_sb[:])
                nc.vector.tensor_sub(b_sb[:], gnb2[:, t:t + 1], t_sb[:])
                ab.append((a_sb, b_sb))
            # normalize slab-major so slab-0 projections start early
            for (m0, mw) in M_SLABS:
                for t in range(2):
                    nc.vector.tensor_scalar(xn_t[t][:, m0:m0 + mw],
                                            x_t[t][:, m0:m0 + mw],
                                            ab[t][0][:], ab[t][1][:],
                                            op0=ALU.mult, op1=ALU.add)

        # ================= Q/K/V projections =================
        with ExitStack() as pctx:
            pps = pctx.enter_context(
                tc.tile_pool(name="proj_ps", bufs=2, space="PSUM"))

            for (m0, mw) in M_SLABS:
                for to in range(2):          # output-channel tile
                    for (w2, b2, dst) in ((wq, bq2, q_t), (wk, bk2, k_t)):
                        ps = pps.tile([128, SLAB], F32, name="qk_ps",
                                      tag="qk", bufs=2)
                        for tch in range(2):
                            nc.tensor.matmul(
                                ps[:, :mw],
                                w2[tch][:, to * 128:(to + 1) * 128],
                                xn_t[tch][:, m0:m0 + mw],
                                start=(tch == 0), stop=(tch == 1))
                        # ACT is idle here; keeps the DVE free for xn
                        # chunks and vT bias adds
                        nc.scalar.activation(
                            dst[to][:, m0:m0 + mw], ps[:, :mw],
                            AF.Identity, bias=b2[:, to:to + 1])

            for tn in range(NT):
                ps = pps.tile([128, VTW], F32, name="vt_ps", tag="vt", bufs=2)
                for tch in range(2):
                    nc.tensor.matmul(
                        ps[:],
                        xn_t[tch][:, tn * 128:(tn + 1) * 128],
                        wv[tch][:],
                        start=(tch == 0), stop=(tch == 1))
                nc.vector.tensor_add(
                    vt_all[:, tn * VTW:(tn + 1) * VTW], ps[:], vbias[:])

        # ================= attention main loop =================
        with ExitStack() as mctx:
            stp = mctx.enter_context(
                tc.tile_pool(name="st_ps", bufs=4, space="PSUM"))
            outp = mctx.enter_context(
                tc.tile_pool(name="out_ps", bufs=3, space="PSUM"))
            rowp = mctx.enter_context(
                tc.tile_pool(name="row_ps", bufs=1, space="PSUM"))
            epool = mctx.enter_context(tc.tile_pool(name="e_sb", bufs=4))
            osb = mctx.enter_context(tc.tile_pool(name="o_sb", bufs=3))
            rsb = mctx.enter_context(tc.tile_pool(name="r_sb", bufs=2))

            PRE = 4  # S^T matmul prefill depth (= st_ps bufs)
            pending_epi = [None]

            for (m0, mw) in M_SLABS:
                st = {}

                def emit_s(t, m0=m0, mw=mw, st=st):
                    st[t] = stp.tile([128, SLAB], F32, name="st", tag="st",
                                     bufs=PRE)
                    for tch in range(2):
                        nc.tensor.matmul(
                            st[t][:, :mw],
                            k_t[tch][:, t * 128:(t + 1) * 128],
                            q_t[tch][:, m0:m0 + mw],
                            start=(tch == 0), stop=(tch == 1))

                oc = [outp.tile([128, SLAB], F32, name=f"oc{c}", tag="oc",
                                bufs=3) for c in range(2)]
                rows = rowp.tile([128, SLAB], F32, name="rows")

                for t in range(PRE):
                    emit_s(t)
                # previous slab's epilogue lands after this slab's S prefill
                # so the PE never sits on the denominator chain
                if pending_epi[0] is not None:
                    pending_epi[0]()
                    pending_epi[0] = None

                etri = []  # pending exp tiles for the 3-way rowsum pre-sum
                for t in range(NT):
                    e_t = epool.tile([128, SLAB], F32R, name="e_t", tag="e",
                                     bufs=6)
                    nc.scalar.activation(e_t[:, :mw], st[t][:, :mw], AF.Exp,
                                         scale=QK_SCALE)
                    st.pop(t)
                    if t + PRE < NT:
                        emit_s(t + PRE)
                    tn_off = t * VTW
                    for c in range(2):
                        nc.tensor.matmul(
                            oc[c][:, :mw],
                            vt_all[:, tn_off + c * 128:tn_off + (c + 1) * 128],
                            e_t[:, :mw],
                            start=(t == 0), stop=(t == NT - 1))
                    # denominators: pre-sum groups of 6 exp tiles on the DVE,
                    # then one rowsum matmul per group. lhsT is all-ones, so
                    # every output partition receives the rowsum -- the
                    # result is already broadcast for the reciprocal.
                    etri.append(e_t)
                    if len(etri) == 6:
                        acc = None
                        for j in range(1, 6):
                            nxt = epool.tile([128, SLAB], F32R, name="ea",
                                             tag=f"ea{j % 2}", bufs=2)
                            nc.vector.tensor_add(
                                nxt[:, :mw],
                                (etri[0] if acc is None else acc)[:, :mw],
                                etri[j][:, :mw])
                            acc = nxt
                        nc.tensor.matmul(
                            rows[:, :mw], onescol[:], acc[:, :mw],
                            start=(t == 5), stop=(t == NT - 1))
                        etri = []

                def epilogue(oc=oc, rows=rows, m0=m0, mw=mw):
                    rec = rsb.tile([128, SLAB], F32, name="rec", tag="rec",
                                   bufs=2)
                    nc.vector.reciprocal_approx_fast(rec[:, :mw],
                                                     rows[:, :mw])
                    for c in range(2):
                        o_sb = osb.tile([128, SLAB], F32, name="o_sb",
                                        tag="o", bufs=3)
                        nc.vector.tensor_mul(o_sb[:, :mw], oc[c][:, :mw],
                                             rec[:, :mw])
                        nc.sync.dma_start(
                            d["out"][c * 128:(c + 1) * 128, m0:m0 + mw],
                            o_sb[:, :mw])

                pending_epi[0] = epilogue

            pending_epi[0]()


def build():
    nc = bacc.Bacc("TRN2", target_bir_lowering=False, debug=False,
                   enable_asserts=True, num_devices=N_CORES)
    d = {
        "x": nc.dram_tensor("x", [C, NPIX], F32, kind="ExternalInput").ap(),
        "wqT": nc.dram_tensor("wqT", [C, C], F32R, kind="ExternalInput").ap(),
        "wkT": nc.dram_tensor("wkT", [C, C], F32R, kind="ExternalInput").ap(),
        "wvT": nc.dram_tensor("wvT", [C, VTW], F32R,
                              kind="ExternalInput").ap(),
        "vbias": nc.dram_tensor("vbias", [128, VTW], F32,
                                kind="ExternalInput").ap(),
        "bq2": nc.dram_tensor("bq2", [128, 2], F32, kind="ExternalInput").ap(),
        "bk2": nc.dram_tensor("bk2", [128, 2], F32, kind="ExternalInput").ap(),
        "gnw2": nc.dram_tensor("gnw2", [128, 2], F32,
                               kind="ExternalInput").ap(),
        "gnb2": nc.dram_tensor("gnb2", [128, 2], F32,
                               kind="ExternalInput").ap(),
        "memb0": nc.dram_tensor("memb0", [128, 128], F32,
                                kind="ExternalInput").ap(),
        "memb1": nc.dram_tensor("memb1", [128, 128], F32,
                                kind="ExternalInput").ap(),
        "membT0": nc.dram_tensor("membT0", [G, 128], F32,
                                 kind="ExternalInput").ap(),
        "membT1": nc.dram_tensor("membT1", [G, 128], F32,
                                 kind="ExternalInput").ap(),
        "onescol": nc.dram_tensor("onescol", [128, 128], F32R,
                                  kind="ExternalInput").ap(),
        "out": nc.dram_tensor("out", [C, NPIX], F32,
                              kind="ExternalOutput").ap(),
    }
    with tile.TileContext(nc) as tc:
        _emit(nc, tc, d)
    nc.compile()
    return nc


def _get_nc():
    if not _NC_CACHE:
        _NC_CACHE.append(build())
    return _NC_CACHE[0]


def _host_prep(input, Wq, bq, Wk, bk, Wv, bv, gn_w, gn_b):
    f32 = np.float32
    xs = np.ascontiguousarray(input.reshape(B, C, NPIX)).astype(f32, copy=False)
    wqT = np.ascontiguousarray(Wq.T).astype(f32, copy=False)
    wkT = np.ascontiguousarray(Wk.T).astype(f32, copy=False)
    wvT = np.ascontiguousarray(Wv.T).astype(f32, copy=False)
    vbias = np.ascontiguousarray(
        np.broadcast_to(bv.astype(f32), (128, VTW)))
    onescol = np.ones((128, 128), f32)
    pairify = lambda v: np.ascontiguousarray(v.astype(f32).reshape(2, 128).T)
    p = np.arange(128)
    g = np.arange(128)
    memb0 = ((g[None, :] == p[:, None] // CPG) & (g[None, :] < G)).astype(f32)
    memb1 = ((g[None, :] == 16 + p[:, None] // CPG)
             & (g[None, :] < G)).astype(f32)
    shared = {
        "wqT": wqT, "wkT": wkT, "wvT": wvT, "vbias": vbias,
        "bq2": pairify(bq), "bk2": pairify(bk),
        "gnw2": pairify(gn_w), "gnb2": pairify(gn_b),
        "memb0": memb0, "memb1": memb1,
        "membT0": np.ascontiguousarray(memb0[:, :G].T),
        "membT1": np.ascontiguousarray(memb1[:, :G].T),
        "onescol": onescol,
    }
    return [{"x": np.ascontiguousarray(xs[c]), **shared}
            for c in range(N_CORES)]


def run(inputs, trace=False):
    nc = _get_nc()
    in_maps = _host_prep(**inputs)
    res = run_bass_kernel_spmd(nc, in_maps, list(range(N_CORES)), trace=trace)
    out = np.stack([res.results[c]["out"] for c in range(N_CORES)])
    return out.reshape(B, C, HH, WW), res


def kernel(**inputs):
    out, _ = run(inputs, trace=False)
    return out


# revision 59
# speedup vs baseline: 1.4790x; 1.0156x over previous
"""AttentionBlock (GroupNorm -> QKV 1x1conv -> 2304x2304 spatial attention)
on 8 Trainium2 NeuronCores, data-parallel over batch.

Per core (one batch element b), with x = input[b] viewed as [C=256, N=2304]:
  gn    = groupnorm(x)                          (32 groups of 8 channels)
  q,k   = Wq@gn, Wk@gn        stored [C, N]     (channel on partitions)
  vT    = gn^T @ Wv^T (+ ones col)  [N, C+1]    (pixel on partitions)
  S^T   = k^T tiles . q       [n-tile, m]       (scores transposed)
  E     = exp(S^T / 16)                         (no max-sub: |scores| ~ 1)
  out   = (vT^T . E) / (ones^T . E)             (rowsum via vT's ones col)

All large matmuls run fp32r (full PE rate at free-dim >= 256).
"""
import numpy as np
from contextlib import ExitStack

import concourse.bass as bass
import concourse.tile as tile
from concourse import bacc, mybir
from concourse.bass_utils import run_bass_kernel_spmd

B, C, HH, WW = 8, 256, 48, 48
NPIX = HH * WW            # 2304
G = 32                    # groups
CPG = C // G              # 8 channels per group
EPS = 1e-5
N_CORES = 8
NT = NPIX // 128          # 18 pixel tiles
SLAB = 512
M_SLABS = [(j, min(SLAB, NPIX - j)) for j in range(0, NPIX, SLAB)]
VTW = C                   # vT slab width (no ones col; see onescol)
INV_GN = 1.0 / (CPG * NPIX)
QK_SCALE = 1.0 / 16.0     # 1/sqrt(C)

F32 = mybir.dt.float32
F32R = mybir.dt.float32r
AF = mybir.ActivationFunctionType
ALU = mybir.AluOpType
AX = mybir.AxisListType

_NC_CACHE = []


def _emit(nc, tc, d):
    with ExitStack() as ctx:
        pers = ctx.enter_context(tc.tile_pool(name="pers", bufs=1))

        # ---- persistent SBUF tensors ----
        x_t = [pers.tile([128, NPIX], F32, name=f"x{t}") for t in range(2)]
        xn_t = [pers.tile([128, NPIX], F32R, name=f"xn{t}") for t in range(2)]
        q_t = [pers.tile([128, NPIX], F32R, name=f"q{t}") for t in range(2)]
        k_t = [pers.tile([128, NPIX], F32R, name=f"k{t}") for t in range(2)]
        vt_all = pers.tile([128, NT * VTW], F32R, name="vt_all")

        wq = [pers.tile([128, C], F32R, name=f"wq{t}") for t in range(2)]
        wk = [pers.tile([128, C], F32R, name=f"wk{t}") for t in range(2)]
        wv = [pers.tile([128, VTW], F32R, name=f"wv{t}") for t in range(2)]
        vbias = pers.tile([128, VTW], F32, name="vbias")
        bq2 = pers.tile([128, 2], F32, name="bq2")
        bk2 = pers.tile([128, 2], F32, name="bk2")
        gnw2 = pers.tile([128, 2], F32, name="gnw2")
        gnb2 = pers.tile([128, 2], F32, name="gnb2")
        memb = [pers.tile([128, 128], F32, name=f"memb{t}") for t in range(2)]
        membT = [pers.tile([G, 128], F32, name=f"membT{t}") for t in range(2)]
        onescol = pers.tile([128, 128], F32R, name="onescol")

        # Everything waits on GN stats, which need the full x: give x the
        # whole HBM bandwidth first (one tile per DMA queue), then the tiny
        # GN constants, then the projection weights (needed a few us later).
        XCH = NPIX // 3  # 768 = two bn_stats windows per chunk
        for h in range(3):
            nc.sync.dma_start(
                x_t[0][:, h * XCH:(h + 1) * XCH],
                d["x"][0:128, h * XCH:(h + 1) * XCH])
            nc.scalar.dma_start(
                x_t[1][:, h * XCH:(h + 1) * XCH],
                d["x"][128:256, h * XCH:(h + 1) * XCH])
        for t in range(2):
            nc.sync.dma_start(memb[t][:], d[f"memb{t}"][:])
            nc.scalar.dma_start(membT[t][:], d[f"membT{t}"][:])
        nc.sync.dma_start(gnw2[:], d["gnw2"][:])
        nc.scalar.dma_start(gnb2[:], d["gnb2"][:])
        for t in range(2):
            nc.sync.dma_start(wq[t][:], d["wqT"][t * 128:(t + 1) * 128, :])
            nc.scalar.dma_start(wk[t][:], d["wkT"][t * 128:(t + 1) * 128, :])
            nc.sync.dma_start(wv[t][:], d["wvT"][t * 128:(t + 1) * 128, :])
        nc.scalar.dma_start(vbias[:], d["vbias"][:])
        nc.sync.dma_start(bq2[:], d["bq2"][:])
        nc.scalar.dma_start(bk2[:], d["bk2"][:])
        nc.scalar.dma_start(onescol[:], d["onescol"][:])

        # ================= GroupNorm =================
        with ExitStack() as pctx:
            sc = pctx.enter_context(tc.tile_pool(name="gn_sc", bufs=2))
            gps = pctx.enter_context(
                tc.tile_pool(name="gn_ps", bufs=2, space="PSUM"))

            # per-partition mean/var in one DVE pass per half via bn_stats
            # (384-wide windows; 2304 = 6 x 384), aggregated by bn_aggr.
            # stats[t] = [mean_ch, E[x^2]_ch]
            BNW = 384
            stats = [sc.tile([128, 2], F32, name=f"stats{t}") for t in range(2)]
            for t in range(2):
                bnraw = sc.tile([128, 36], F32, name="bnraw", tag="bnr",
                                bufs=2)
                for w in range(6):
                    nc.vector.bn_stats(
                        bnraw[:, 6 * w:6 * w + 6],
                        x_t[t][:, w * BNW:(w + 1) * BNW])
                mv = sc.tile([128, 2], F32, name="mv", tag="mv", bufs=2)
                nc.vector.bn_aggr(mv[:], bnraw[:])
                m2 = sc.tile([128, 1], F32, name="m2", tag="m2", bufs=2)
                nc.vector.tensor_mul(m2[:], mv[:, 0:1], mv[:, 0:1])
                nc.vector.tensor_copy(stats[t][:, 0:1], mv[:, 0:1])
                nc.vector.tensor_add(stats[t][:, 1:2], mv[:, 1:2], m2[:])

            # group sums: partitions 0..31 of memb^T stats (memb cols 32..127
            # are zero padding -- fp32 matmul needs full col groups)
            g_ps = gps.tile([128, 2], F32, name="g_ps")
            for t in range(2):
                nc.tensor.matmul(g_ps[:], memb[t][:], stats[t][:],
                                 start=(t == 0), stop=(t == 1))

            mr = sc.tile([G, 2], F32, name="mr")          # [mean, rstd]
            tmp1 = sc.tile([G, 1], F32, name="tmp1")
            tmp2 = sc.tile([G, 1], F32, name="tmp2")
            tmp3 = sc.tile([G, 1], F32, name="tmp3")
            nc.vector.tensor_scalar_mul(mr[:, 0:1], g_ps[0:G, 0:1], 1.0 / CPG)
            nc.vector.tensor_scalar_mul(tmp1[:], g_ps[0:G, 1:2], 1.0 / CPG)
            nc.vector.tensor_mul(tmp2[:], mr[:, 0:1], mr[:, 0:1])
            nc.vector.tensor_sub(tmp3[:], tmp1[:], tmp2[:])       # var
            nc.vector.tensor_scalar_add(tmp3[:], tmp3[:], EPS)
            nc.scalar.activation(tmp1[:], tmp3[:], AF.Sqrt)
            nc.vector.reciprocal_approx_fast(mr[:, 1:2], tmp1[:])  # rstd

            # broadcast per-group (mean, rstd) back to channels; fold gn w/b
            ab = []
            for t in range(2):
                ch_ps = gps.tile([128, 2], F32, name="ch_ps", tag="ch", bufs=2)
                nc.tensor.matmul(ch_ps[:], membT[t][:], mr[:],
                                 start=True, stop=True)
                a_sb = sc.tile([128, 1], F32, name="a_sb", tag="a", bufs=2)
                t_sb = sc.tile([128, 1], F32, name="t_sb", tag="t", bufs=2)
                b_sb = sc.tile([128, 1], F32, name="b_sb", tag="b", bufs=2)
                nc.vector.tensor_mul(a_sb[:], ch_ps[:, 1:2], gnw2[:, t:t + 1])
                nc.vector.tensor_mul(t_sb[:], ch_ps[:, 0:1], a_sb[:])
                nc.vector.tensor_sub(b_sb[:], gnb2[:, t:t + 1], t_sb[:])
                ab.append((a_sb, b_sb))
            # normalize slab-major so slab-0 projections start early
            for (m0, mw) in M_SLABS:
                for t in range(2):
                    nc.vector.tensor_scalar(xn_t[t][:, m0:m0 + mw],
                                            x_t[t][:, m0:m0 + mw],
                                            ab[t][0][:], ab[t][1][:],
                                            op0=ALU.mult, op1=ALU.add)

        # ================= Q/K/V projections =================
        with ExitStack() as pctx:
            pps = pctx.enter_context(
                tc.tile_pool(name="proj_ps", bufs=2, space="PSUM"))

            def emit_vt(tn):
                ps = pps.tile([128, VTW], F32, name="vt_ps", tag="vt", bufs=2)
                for tch in range(2):
                    nc.tensor.matmul(
                        ps[:],
                        xn_t[tch][:, tn * 128:(tn + 1) * 128],
                        wv[tch][:],
                        start=(tch == 0), stop=(tch == 1))
                nc.vector.tensor_add(
                    vt_all[:, tn * VTW:(tn + 1) * VTW], ps[:], vbias[:])

            # interleave vT (N=256) with q/k (N=512) matmuls so the vT
            # weight loads hide behind the longer q/k streams
            vt_done = 0
            for si, (m0, mw) in enumerate(M_SLABS):
                for to in range(2):          # output-channel tile
                    for (w2, b2, dst) in ((wq, bq2, q_t), (wk, bk2, k_t)):
                        ps = pps.tile([128, SLAB], F32, name="qk_ps",
                                      tag="qk", bufs=2)
                        for tch in range(2):
                            nc.tensor.matmul(
                                ps[:, :mw],
                                w2[tch][:, to * 128:(to + 1) * 128],
                                xn_t[tch][:, m0:m0 + mw],
                                start=(tch == 0), stop=(tch == 1))
                        # ACT is idle here; keeps the DVE free for xn
                        # chunks and vT bias adds
                        nc.scalar.activation(
                            dst[to][:, m0:m0 + mw], ps[:, :mw],
                            AF.Identity, bias=b2[:, to:to + 1])
                    vt_avail = min(NT, (m0 + mw) // 128)
                    if vt_done < vt_avail:
                        emit_vt(vt_done)
                        vt_done += 1
            while vt_done < NT:
                emit_vt(vt_done)
                vt_done += 1

        # ================= attention main loop =================
        with ExitStack() as mctx:
            stp = mctx.enter_context(
                tc.tile_pool(name="st_ps", bufs=4, space="PSUM"))
            outp = mctx.enter_context(
                tc.tile_pool(name="out_ps", bufs=3, space="PSUM"))
            rowp = mctx.enter_context(
                tc.tile_pool(name="row_ps", bufs=1, space="PSUM"))
            epool = mctx.enter_context(tc.tile_pool(name="e_sb", bufs=4))
            osb = mctx.enter_context(tc.tile_pool(name="o_sb", bufs=3))
            rsb = mctx.enter_context(tc.tile_pool(name="r_sb", bufs=2))

            PRE = 4  # S^T matmul prefill depth (= st_ps bufs)
            pending_epi = [None]

            for (m0, mw) in M_SLABS:
                st = {}

                def emit_s(t, m0=m0, mw=mw, st=st):
                    st[t] = stp.tile([128, SLAB], F32, name="st", tag="st",
                                     bufs=PRE)
                    for tch in range(2):
                        nc.tensor.matmul(
                            st[t][:, :mw],
                            k_t[tch][:, t * 128:(t + 1) * 128],
                            q_t[tch][:, m0:m0 + mw],
                            start=(tch == 0), stop=(tch == 1))

                oc = [outp.tile([128, SLAB], F32, name=f"oc{c}", tag="oc",
                                bufs=3) for c in range(2)]
                rows = rowp.tile([128, SLAB], F32, name="rows")

                for t in range(PRE):
                    emit_s(t)
                # previous slab's epilogue lands after this slab's S prefill
                # so the PE never sits on the denominator chain
                if pending_epi[0] is not None:
                    pending_epi[0]()
                    pending_epi[0] = None

                etri = []  # pending exp tiles for the 3-way rowsum pre-sum
                for t in range(NT):
                    e_t = epool.tile([128, SLAB], F32R, name="e_t", tag="e",
                                     bufs=6)
                    nc.scalar.activation(e_t[:, :mw], st[t][:, :mw], AF.Exp,
                                         scale=QK_SCALE)
                    st.pop(t)
                    if t + PRE < NT:
                        emit_s(t + PRE)
                    tn_off = t * VTW
                    for c in range(2):
                        nc.tensor.matmul(
                            oc[c][:, :mw],
                            vt_all[:, tn_off + c * 128:tn_off + (c + 1) * 128],
                            e_t[:, :mw],
                            start=(t == 0), stop=(t == NT - 1))
                    # denominators: pre-sum groups of 6 exp tiles on the DVE,
                    # then one rowsum matmul per group. lhsT is all-ones, so
                    # every output partition receives the rowsum -- the
                    # result is already broadcast for the reciprocal.
                    etri.append(e_t)
                    if len(etri) == 6:
                        acc = None
                        for j in range(1, 6):
                            nxt = epool.tile([128, SLAB], F32R, name="ea",
                                             tag=f"ea{j % 2}", bufs=2)
                            nc.vector.tensor_add(
                                nxt[:, :mw],
                                (etri[0] if acc is None else acc)[:, :mw],
                                etri[j][:, :mw])
                            acc = nxt
                        nc.tensor.matmul(
                            rows[:, :mw], onescol[:], acc[:, :mw],
                            start=(t == 5), stop=(t == NT - 1))
                        etri = []

                def epilogue(oc=oc, rows=rows, m0=m0, mw=mw):
                    rec = rsb.tile([128, SLAB], F32, name="rec", tag="rec",
                                   bufs=2)
                    nc.vector.reciprocal_approx_fast(rec[:, :mw],
                                                     rows[:, :mw])
                    for c in range(2):
                        o_sb = osb.tile([128, SLAB], F32, name="o_sb",
                                        tag="o", bufs=3)
                        nc.vector.tensor_mul(o_sb[:, :mw], oc[c][:, :mw],
                                             rec[:, :mw])
                        nc.sync.dma_start(
                            d["out"][c * 128:(c + 1) * 128, m0:m0 + mw],
                            o_sb[:, :mw])

                pending_epi[0] = epilogue

            pending_epi[0]()


def build():
    nc = bacc.Bacc("TRN2", target_bir_lowering=False, debug=False,
                   enable_asserts=True, num_devices=N_CORES)
    d = {
        "x": nc.dram_tensor("x", [C, NPIX], F32, kind="ExternalInput").ap(),
        "wqT": nc.dram_tensor("wqT", [C, C], F32R, kind="ExternalInput").ap(),
        "wkT": nc.dram_tensor("wkT", [C, C], F32R, kind="ExternalInput").ap(),
        "wvT": nc.dram_tensor("wvT", [C, VTW], F32R,
                              kind="ExternalInput").ap(),
        "vbias": nc.dram_tensor("vbias", [128, VTW], F32,
                                kind="ExternalInput").ap(),
        "bq2": nc.dram_tensor("bq2", [128, 2], F32, kind="ExternalInput").ap(),
        "bk2": nc.dram_tensor("bk2", [128, 2], F32, kind="ExternalInput").ap(),
        "gnw2": nc.dram_tensor("gnw2", [128, 2], F32,
                               kind="ExternalInput").ap(),
        "gnb2": nc.dram_tensor("gnb2", [128, 2], F32,
                               kind="ExternalInput").ap(),
        "memb0": nc.dram_tensor("memb0", [128, 128], F32,
                                kind="ExternalInput").ap(),
        "memb1": nc.dram_tensor("memb1", [128, 128], F32,
                                kind="ExternalInput").ap(),
        "membT0": nc.dram_tensor("membT0", [G, 128], F32,
                                 kind="ExternalInput").ap(),
        "membT1": nc.dram_tensor("membT1", [G, 128], F32,
                                 kind="ExternalInput").ap(),
        "onescol": nc.dram_tensor("onescol", [128, 128], F32R,
                                  kind="ExternalInput").ap(),
        "out": nc.dram_tensor("out", [C, NPIX], F32,
                              kind="ExternalOutput").ap(),
    }
    with tile.TileContext(nc) as tc:
        _emit(nc, tc, d)
    nc.compile()
    return nc


def _get_nc():
    if not _NC_CACHE:
        _NC_CACHE.append(build())
    return _NC_CACHE[0]


def _host_prep(input, Wq, bq, Wk, bk, Wv, bv, gn_w, gn_b):
    f32 = np.float32
    xs = np.ascontiguousarray(input.reshape(B, C, NPIX)).astype(f32, copy=False)
    wqT = np.ascontiguousarray(Wq.T).astype(f32, copy=False)
    wkT = np.ascontiguousarray(Wk.T).astype(f32, copy=False)
    wvT = np.ascontiguousarray(Wv.T).astype(f32, copy=False)
    vbias = np.ascontiguousarray(
        np.broadcast_to(bv.astype(f32), (128, VTW)))
    onescol = np.ones((128, 128), f32)
    pairify = lambda v: np.ascontiguousarray(v.astype(f32).reshape(2, 128).T)
    p = np.arange(128)
    g = np.arange(128)
    memb0 = ((g[None, :] == p[:, None] // CPG) & (g[None, :] < G)).astype(f32)
    memb1 = ((g[None, :] == 16 + p[:, None] // CPG)
             & (g[None, :] < G)).astype(f32)
    shared = {
        "wqT": wqT, "wkT": wkT, "wvT": wvT, "vbias": vbias,
        "bq2": pairify(bq), "bk2": pairify(bk),
        "gnw2": pairify(gn_w), "gnb2": pairify(gn_b),
        "memb0": memb0, "memb1": memb1,
        "membT0": np.ascontiguousarray(memb0[:, :G].T),
        "membT1": np.ascontiguousarray(memb1[:, :G].T),
        "onescol": onescol,
    }
    return [{"x": np.ascontiguousarray(xs[c]), **shared}
            for c in range(N_CORES)]


def run(inputs, trace=False):
    nc = _get_nc()
    in_maps = _host_prep(**inputs)
    res = run_bass_kernel_spmd(nc, in_maps, list(range(N_CORES)), trace=trace)
    out = np.stack([res.results[c]["out"] for c in range(N_CORES)])
    return out.reshape(B, C, HH, WW), res


def kernel(**inputs):
    out, _ = run(inputs, trace=False)
    return out
